# revision 13
# baseline (speedup 1.0000x reference)
"""DistSAGE 3-layer GraphSAGE forward on 8 TRN2 NeuronCores (Bass/Tile).

Strategy (graph/data parallel, per the DistSAGE recipe):
  - Partition the 512 seed nodes across 8 cores (64 each); build per-core
    dependency-driven blocks on the host (standard DGL block construction,
    pure index math): layer-2 dsts = seeds, layer-1 dsts = seeds + their
    layer-2 sources, layer-0 dsts = layer-1 dsts + their layer-1 sources.
    No inter-core communication; weights replicated.
  - Row-shard the feature table: each core gets a compact bf16 table with
    only the x rows its block touches, laid out in per-dst-tile "bands" so
    each dma_gather call addresses rows with int16 indices relative to a
    32768-row window (the gather ucode's index width).
  - Per 128-dst tile on device: dma_gather the tile's unique source rows
    (128/chunk, up to 4096/call), accumulate meanT[f,d] += msgs.T @ S' on
    the TensorEngine in PSUM, where S'[slot,d] = multiplicity(slot->d)/deg[d]
    is a host-precomputed bf16 matrix DMA'd on the (otherwise idle) HWDGE
    line.  A per-tile "self chunk" whose S' is the identity yields h_dstT
    the same way (doubling as the transpose).  Then Y[d,:] =
    meanT.T@W_neigh + h_dstT.T@W_self (4 PSUM-accumulated bf16 matmuls),
    + bias, ReLU, DMA the bf16 tile to DRAM for the next layer's gather.
"""

import heapq

import numpy as np

P = 128
NCORES = 8
NUM_DST = (61952, 5632, 512)
FEAT = 256
OUTW = (256, 256, 19)
SEEDS_PER_CORE = NUM_DST[2] // NCORES  # 64
WINDOW = 32768
NI_L = (4096, 4096, 1024)  # dma_gather indices per call, per layer


def _bf16():
    import ml_dtypes

    return ml_dtypes.bfloat16


# ---------------------------------------------------------------------------
# Host-side block construction
# ---------------------------------------------------------------------------


def _balance(ids, deg, n_buckets):
    """LPT bin-packing: reorder ids so consecutive 128-groups have ~equal
    total degree. len(ids) must be >= n_buckets * 128 is not required; only
    full groups are balanced."""
    if n_buckets <= 1 or len(ids) < n_buckets * P:
        return ids
    order = np.argsort(-deg[ids], kind="stable")
    heap = [(0.0, b, 0) for b in range(n_buckets)]
    heapq.heapify(heap)
    buckets = [[] for _ in range(n_buckets)]
    for i in order:
        load, b, cnt = heapq.heappop(heap)
        buckets[b].append(ids[i])
        cnt += 1
        if cnt < P:
            heapq.heappush(heap, (load + deg[ids[i]], b, cnt))
    return np.concatenate([np.asarray(b, dtype=ids.dtype) for b in buckets])


def _block_for_core(c, esrc0, edst0, esrc1, edst1, esrc2, edst2, deg0, deg1, deg2):
    seeds = np.arange(c * SEEDS_PER_CORE, (c + 1) * SEEDS_PER_CORE, dtype=np.int64)

    sel2 = (edst2 >= seeds[0]) & (edst2 < seeds[0] + SEEDS_PER_CORE)
    es2, ed2g = esrc2[sel2], edst2[sel2]
    l1_extra = np.setdiff1d(np.unique(es2), seeds)
    nfull = (len(l1_extra) // P) * P
    if nfull >= P:
        l1_extra = np.concatenate(
            [_balance(l1_extra[:nfull], deg1, nfull // P), l1_extra[nfull:]]
        )
    l1_out = np.concatenate([seeds, l1_extra])
    n1 = len(l1_out)

    pos1 = np.full(NUM_DST[1], -1, np.int32)
    pos1[l1_out] = np.arange(n1, dtype=np.int32)
    sel1 = pos1[edst1] >= 0
    es1, ed1g = esrc1[sel1], edst1[sel1]
    ed1 = pos1[ed1g].astype(np.int64)
    inv1 = (1.0 / np.maximum(deg1[ed1g], 1.0)).astype(np.float32)
    l0_extra = np.setdiff1d(np.unique(es1), l1_out)
    nfull = (len(l0_extra) // P) * P
    if nfull >= P:
        l0_extra = np.concatenate(
            [_balance(l0_extra[:nfull], deg0, nfull // P), l0_extra[nfull:]]
        )
    l0_out = np.concatenate([l1_out, l0_extra])
    n0 = len(l0_out)

    pos0 = np.full(NUM_DST[0], -1, np.int32)
    pos0[l0_out] = np.arange(n0, dtype=np.int32)
    sel0 = pos0[edst0] >= 0
    es0, ed0g = esrc0[sel0], edst0[sel0]
    ed0 = pos0[ed0g].astype(np.int64)
    inv0 = (1.0 / np.maximum(deg0[ed0g], 1.0)).astype(np.float32)

    ed2 = pos1[ed2g].astype(np.int64)
    inv2 = (1.0 / np.maximum(deg2[ed2g], 1.0)).astype(np.float32)
    es2l = pos1[es2].astype(np.int64)
    es1l = pos0[es1].astype(np.int64)

    return dict(
        l1_out=l1_out, l0_out=l0_out, n1=n1, n0=n0,
        e0=(es0.astype(np.int64), ed0, inv0),
        e1=(es1l, ed1, inv1),
        e2=(es2l, ed2, inv2),
    )


def _group_edges_by_tile(es, ed, inv, n_tiles):
    """Per dst-tile: dedup sources and build the dense S' payload.

    Returns per-tile (unique_srcs sorted, W [n_u, 128] f32) where
    W[slot, d] = sum of invdeg over edges (src_slot -> tile_base + d)."""
    tile = ed // P
    order = np.argsort(tile, kind="stable")
    es, ed, inv, tile = es[order], ed[order], inv[order], tile[order]
    starts = np.searchsorted(tile, np.arange(n_tiles))
    ends = np.searchsorted(tile, np.arange(n_tiles) + 1)
    out = []
    for t in range(n_tiles):
        s, e = starts[t], ends[t]
        u, ii = np.unique(es[s:e], return_inverse=True)
        W = np.zeros((len(u), P), np.float32)
        np.add.at(W, (ii, ed[s:e] - t * P), inv[s:e])
        out.append((u, W))
    return out


class LayerPlan:
    def __init__(self, n_tiles, chunks_per_tile, ni):
        self.ni = ni
        self.cpc = ni // P
        self.n_tiles = n_tiles
        self.chunks_per_tile = chunks_per_tile
        self.col_self = []
        self.col_edges = []
        col = 0
        for t in range(n_tiles):
            self.col_self.append(col)
            col += 1
            self.col_edges.append(list(range(col, col + chunks_per_tile[t])))
            col += chunks_per_tile[t]
        self.n_cols = col
        self.n_calls = -(-col // self.cpc)
        self.n_cols_pad = self.n_calls * self.cpc
        self.gidx = []  # [NCORES][128, n_cols_pad] int64 table rows
        self.wmat = []  # [NCORES][128, n_cols_pad, 128] f32 S' content
        self.call_base = None  # [n_calls] int64, uniform across cores


def _plan_layer(per_core_tiles, n_tiles, ni):
    chunks = [
        max(1, max(-(-len(per_core_tiles[c][t][0]) // P) for c in range(NCORES)))
        for t in range(n_tiles)
    ]
    return LayerPlan(n_tiles, chunks, ni)


def _fill_plan(plan, per_core_tiles, self_rows, pad_row):
    for c in range(NCORES):
        gidx = np.zeros((P, plan.n_cols_pad), np.int64)
        wmat = np.zeros((P, plan.n_cols_pad, P), np.float32)
        eye = np.eye(P, dtype=np.float32)
        for t in range(plan.n_tiles):
            gidx[:, plan.col_self[t]] = self_rows[c][t]
            wmat[:, plan.col_self[t], :] = eye
            u, W = per_core_tiles[c][t]
            n = len(u)
            cols = plan.col_edges[t]
            bi = np.full(len(cols) * P, pad_row[c][t], np.int64)
            bw = np.zeros((len(cols) * P, P), np.float32)
            bi[:n] = u
            bw[:n] = W
            for j, col in enumerate(cols):
                gidx[:, col] = bi[j * P : (j + 1) * P]
                wmat[:, col, :] = bw[j * P : (j + 1) * P]
        if plan.n_cols_pad > plan.n_cols:
            gidx[:, plan.n_cols :] = pad_row[c][plan.n_tiles - 1]
        plan.gidx.append(gidx)
        plan.wmat.append(wmat)


def build_host(inputs):
    esrc0 = np.asarray(inputs["esrc0"]).astype(np.int64)
    edst0 = np.asarray(inputs["edst0"]).astype(np.int64)
    esrc1 = np.asarray(inputs["esrc1"]).astype(np.int64)
    edst1 = np.asarray(inputs["edst1"]).astype(np.int64)
    esrc2 = np.asarray(inputs["esrc2"]).astype(np.int64)
    edst2 = np.asarray(inputs["edst2"]).astype(np.int64)
    x = np.asarray(inputs["x"], dtype=np.float32)

    deg0 = np.bincount(edst0, minlength=NUM_DST[0]).astype(np.float32)
    deg1 = np.bincount(edst1, minlength=NUM_DST[1]).astype(np.float32)
    deg2 = np.bincount(edst2, minlength=NUM_DST[2]).astype(np.float32)

    blocks = [
        _block_for_core(c, esrc0, edst0, esrc1, edst1, esrc2, edst2, deg0, deg1, deg2)
        for c in range(NCORES)
    ]

    n0_pad = max(-(-b["n0"] // P) for b in blocks) * P
    n1_pad = max(-(-b["n1"] // P) for b in blocks) * P
    T0, T1, T2 = n0_pad // P, n1_pad // P, 1

    tiles0 = [_group_edges_by_tile(*b["e0"], T0) for b in blocks]
    tiles1 = [_group_edges_by_tile(*b["e1"], T1) for b in blocks]
    tiles2 = [_group_edges_by_tile(*b["e2"], T2) for b in blocks]

    plan0 = _plan_layer(tiles0, T0, NI_L[0])
    plan1 = _plan_layer(tiles1, T1, NI_L[1])
    plan2 = _plan_layer(tiles2, T2, NI_L[2])

    l0_padded = []
    for b in blocks:
        v = np.zeros(T0 * P, np.int64)
        v[: b["n0"]] = b["l0_out"]
        v[b["n0"] :] = b["l0_out"][0]
        l0_padded.append(v)
    self0 = [
        [l0_padded[c][t * P : (t + 1) * P] for t in range(T0)] for c in range(NCORES)
    ]
    pad0 = [[l0_padded[c][t * P] for t in range(T0)] for c in range(NCORES)]
    _fill_plan(plan0, tiles0, self0, pad0)

    ar = np.arange(P, dtype=np.int64)
    selfL = lambda T: [[t * P + ar for t in range(T)] for _ in range(NCORES)]
    padL = lambda T: [[t * P for t in range(T)] for _ in range(NCORES)]
    _fill_plan(plan1, tiles1, selfL(T1), padL(T1))
    _fill_plan(plan2, tiles2, selfL(T2), padL(T2))

    # ---- layer 0: banded compact x table (uniform band offsets) ----
    bands = []
    for c in range(NCORES):
        bands.append(
            [
                np.unique(plan0.gidx[c][:, [plan0.col_self[t]] + plan0.col_edges[t]])
                for t in range(T0)
            ]
        )
    band_size = np.array(
        [max(len(bands[c][t]) for c in range(NCORES)) for t in range(T0)], np.int64
    )
    band_start = np.concatenate([[0], np.cumsum(band_size)])

    for c in range(NCORES):
        gidx, new = plan0.gidx[c], np.zeros_like(plan0.gidx[c])
        for t in range(T0):
            band = bands[c][t]
            cols = [plan0.col_self[t]] + plan0.col_edges[t]
            sl = gidx[:, cols]
            loc = np.searchsorted(band, sl)
            assert (band[loc] == sl).all()
            new[:, cols] = band_start[t] + loc
        if plan0.n_cols_pad > plan0.n_cols:
            band = bands[c][T0 - 1]
            sl = gidx[:, plan0.n_cols :]
            new[:, plan0.n_cols :] = band_start[T0 - 1] + np.searchsorted(band, sl)
        plan0.gidx[c] = new

    col_tile = np.zeros(plan0.n_cols_pad, np.int64)
    for t in range(T0):
        for col in [plan0.col_self[t]] + plan0.col_edges[t]:
            col_tile[col] = t
    col_tile[plan0.n_cols :] = T0 - 1
    cpc0 = plan0.cpc
    plan0.call_base = np.array(
        [band_start[col_tile[k * cpc0]] for k in range(plan0.n_calls)], np.int64
    )
    for c in range(NCORES):
        for k in range(plan0.n_calls):
            sl = plan0.gidx[c][:, k * cpc0 : (k + 1) * cpc0]
            assert sl.min() >= plan0.call_base[k], (c, k)
            assert sl.max() < plan0.call_base[k] + WINDOW, (c, k)
    plan1.call_base = np.zeros(plan1.n_calls, np.int64)
    plan2.call_base = np.zeros(plan2.n_calls, np.int64)
    assert n0_pad <= WINDOW and n1_pad <= WINDOW

    bf16 = _bf16()
    x16 = x.astype(bf16)
    xc_len_pad = -(-int(band_start[T0]) // P) * P
    xcs = []
    for c in range(NCORES):
        t = np.zeros((xc_len_pad, FEAT), bf16)
        for tt in range(T0):
            b = bands[c][tt]
            t[band_start[tt] : band_start[tt] + len(b)] = x16[b]
        xcs.append(t)

    return dict(
        plans=(plan0, plan1, plan2),
        T=(T0, T1, T2),
        n0_pad=n0_pad,
        n1_pad=n1_pad,
        xc_len_pad=xc_len_pad,
        xcs=xcs,
        blocks=blocks,
        weights=tuple(
            (
                np.asarray(inputs[f"W_self{l}"], np.float32),
                np.asarray(inputs[f"W_neigh{l}"], np.float32),
                np.asarray(inputs[f"b{l}"], np.float32),
            )
            for l in range(3)
        ),
    )


# ---------------------------------------------------------------------------
# Numpy simulation of the device kernel (validation aid; fp32 stand-in)
# ---------------------------------------------------------------------------


def simulate_core(meta, c):
    table = meta["xcs"][c].astype(np.float32)
    for l, plan in enumerate(meta["plans"]):
        ws, wn, b = meta["weights"][l]
        out = np.zeros((plan.n_tiles * P, OUTW[l]), np.float32)
        for t in range(plan.n_tiles):
            hd = table[plan.gidx[c][:, plan.col_self[t]]]
            aggT = np.zeros((FEAT, P), np.float32)
            for col in plan.col_edges[t]:
                msgs = table[plan.gidx[c][:, col]]
                aggT += msgs.T @ plan.wmat[c][:, col, :]
            y = hd @ ws + aggT.T @ wn + b
            if l < 2:
                y = np.maximum(y, 0.0)
            out[t * P : (t + 1) * P] = y
        table = out
    return table[:SEEDS_PER_CORE]


# ---------------------------------------------------------------------------
# Device kernel
# ---------------------------------------------------------------------------


def _wrap_idx16(plan, c):
    """Relative rows -> dma_gather idx layout [128, n_calls*ni/16] int16
    (16-partition wrap, replicated x8)."""
    ni, cpc = plan.ni, plan.cpc
    rel = plan.gidx[c] - np.repeat(plan.call_base, cpc)[None, :]
    n_calls = plan.n_calls
    out = np.zeros((P, n_calls * ni // 16), np.int16)
    for k in range(n_calls):
        flat = rel[:, k * cpc : (k + 1) * cpc].T.reshape(-1)  # i = j*128+p
        w = flat.reshape(ni // 16, 16).T.astype(np.int16)
        out[:16, k * (ni // 16) : (k + 1) * (ni // 16)] = w
    for rep in range(1, 8):
        out[rep * 16 : (rep + 1) * 16] = out[:16]
    return out


def run_device(meta, trace=False):
    import concourse.bacc as bacc
    import concourse.tile as tile
    import concourse.mybir as mybir
    from concourse.bass_utils import run_bass_kernel_spmd

    plans = meta["plans"]
    f32 = mybir.dt.float32
    b16 = mybir.dt.bfloat16

    nc = bacc.Bacc("TRN2", target_bir_lowering=False, debug=False, num_devices=NCORES)

    xc = nc.dram_tensor("xc", [meta["xc_len_pad"], FEAT], b16, kind="ExternalInput")
    h1buf = nc.dram_tensor("h1buf", [meta["n0_pad"], FEAT], b16)
    h2buf = nc.dram_tensor("h2buf", [meta["n1_pad"], FEAT], b16)
    out_d = nc.dram_tensor("out", [SEEDS_PER_CORE, OUTW[2]], f32, kind="ExternalOutput")

    idx_d, sp_d, w_d = [], [], []
    for l, plan in enumerate(plans):
        idx_d.append(
            nc.dram_tensor(f"gidx{l}", [P, plan.n_calls * plan.ni // 16],
                           mybir.dt.int16, kind="ExternalInput")
        )
        sp_d.append(
            nc.dram_tensor(f"sp{l}", [P, plan.n_cols_pad * P], b16,
                           kind="ExternalInput")
        )
        w_d.append(
            (
                nc.dram_tensor(f"ws{l}", [FEAT, OUTW[l]], b16, kind="ExternalInput"),
                nc.dram_tensor(f"wn{l}", [FEAT, OUTW[l]], b16, kind="ExternalInput"),
                nc.dram_tensor(f"bias{l}", [P, OUTW[l]], f32, kind="ExternalInput"),
            )
        )

    tables = [xc, h1buf, h2buf]
    dests = [h1buf, h2buf, out_d]

    with tile.TileContext(nc) as tc:
        with (
            tc.tile_pool(name="const", bufs=1) as cpool,
            tc.tile_pool(name="msgs", bufs=3) as mpool,
            tc.tile_pool(name="sel", bufs=3) as spool,
            tc.tile_pool(name="acc", bufs=2) as apool,
            tc.tile_pool(name="outp", bufs=3) as opool,
            tc.tile_pool(name="pagg", bufs=1, space="PSUM") as pa,
            tc.tile_pool(name="py", bufs=2, space="PSUM") as pypool,
        ):
            # preload every layer's constants up front (HWDGE line is idle
            # at kernel start; keeps layer transitions seamless)
            idx_ts, ws_ts, wn_ts, bias_ts = [], [], [], []
            for l, plan in enumerate(plans):
                outw = OUTW[l]
                idx_t = cpool.tile(list(idx_d[l].shape), mybir.dt.int16, tag=f"idx{l}")
                nc.sync.dma_start(out=idx_t[:], in_=idx_d[l][:])
                idx_ts.append(idx_t)
                wst, wnt = [], []
                for k in range(2):
                    w = cpool.tile([P, outw], b16, tag=f"ws{l}_{k}")
                    nc.sync.dma_start(out=w[:], in_=w_d[l][0][k * P : (k + 1) * P, :])
                    wst.append(w)
                    w = cpool.tile([P, outw], b16, tag=f"wn{l}_{k}")
                    nc.sync.dma_start(out=w[:], in_=w_d[l][1][k * P : (k + 1) * P, :])
                    wnt.append(w)
                ws_ts.append(wst)
                wn_ts.append(wnt)
                bias_t = cpool.tile([P, outw], f32, tag=f"bias{l}")
                nc.sync.dma_start(out=bias_t[:], in_=w_d[l][2][:])
                bias_ts.append(bias_t)

            for l, plan in enumerate(plans):
                outw = OUTW[l]
                table, dest = tables[l], dests[l]
                ws_t, wn_t, bias_t, idx_t = ws_ts[l], wn_ts[l], bias_ts[l], idx_ts[l]
                ni, cpc = plan.ni, plan.cpc

                call_tiles, sp_tiles = [], []
                for k in range(plan.n_calls):
                    mt = mpool.tile([P, cpc * FEAT], b16, tag="msgs")
                    base = int(plan.call_base[k])
                    hi = min(base + WINDOW, table.shape[0])
                    nc.gpsimd.dma_gather(
                        out_ap=mt[:, : cpc * FEAT].rearrange(
                            "p (g d) -> p g d", g=cpc
                        ),
                        in_ap=table[base:hi, :],
                        idxs_ap=idx_t[:, k * (ni // 16) : (k + 1) * (ni // 16)],
                        num_idxs=ni,
                        num_idxs_reg=ni,
                        elem_size=FEAT,
                        single_packet=False,
                    )
                    call_tiles.append(mt)
                    st = spool.tile([P, cpc * P], b16, tag="sp")
                    nc.sync.dma_start(
                        out=st[:, : cpc * P],
                        in_=sp_d[l][:, k * cpc * P : (k + 1) * cpc * P],
                    )
                    sp_tiles.append(st)

                def msg_slice(col, f0, f1):
                    k, j = divmod(col, cpc)
                    return call_tiles[k][:, j * FEAT + f0 : j * FEAT + f1]

                def sp_slice(col):
                    k, j = divmod(col, cpc)
                    return sp_tiles[k][:, j * P : (j + 1) * P]

                for t in range(plan.n_tiles):
                    cs = plan.col_self[t]
                    ph0 = pa.tile([P, P], f32, tag="ph0")
                    ph1 = pa.tile([P, P], f32, tag="ph1")
                    nc.tensor.matmul(ph0[:], lhsT=msg_slice(cs, 0, P),
                                     rhs=sp_slice(cs), start=True, stop=True)
                    nc.tensor.matmul(ph1[:], lhsT=msg_slice(cs, P, 2 * P),
                                     rhs=sp_slice(cs), start=True, stop=True)
                    pa0 = pa.tile([P, P], f32, tag="pa0")
                    pa1 = pa.tile([P, P], f32, tag="pa1")
                    cols = plan.col_edges[t]
                    for i, col in enumerate(cols):
                        st, sp = (i == 0), (i == len(cols) - 1)
                        nc.tensor.matmul(pa0[:], lhsT=msg_slice(col, 0, P),
                                         rhs=sp_slice(col), start=st, stop=sp)
                        nc.tensor.matmul(pa1[:], lhsT=msg_slice(col, P, 2 * P),
                                         rhs=sp_slice(col), start=st, stop=sp)
                    a0 = apool.tile([P, P], b16, tag="a0")
                    nc.vector.tensor_copy(out=a0[:], in_=pa0[:])
                    a1 = apool.tile([P, P], b16, tag="a1")
                    nc.vector.tensor_copy(out=a1[:], in_=pa1[:])
                    h0 = apool.tile([P, P], b16, tag="h0")
                    nc.vector.tensor_copy(out=h0[:], in_=ph0[:])
                    h1 = apool.tile([P, P], b16, tag="h1")
                    nc.vector.tensor_copy(out=h1[:], in_=ph1[:])
                    y = pypool.tile([P, outw], f32, tag="y")
                    nc.tensor.matmul(y[:], lhsT=a0[:], rhs=wn_t[0][:],
                                     start=True, stop=False)
                    nc.tensor.matmul(y[:], lhsT=a1[:], rhs=wn_t[1][:],
                                     start=False, stop=False)
                    nc.tensor.matmul(y[:], lhsT=h0[:], rhs=ws_t[0][:],
                                     start=False, stop=False)
                    nc.tensor.matmul(y[:], lhsT=h1[:], rhs=ws_t[1][:],
                                     start=False, stop=True)
                    o = opool.tile([P, outw], f32, tag="o")
                    nc.vector.tensor_tensor(out=o[:], in0=y[:], in1=bias_t[:],
                                            op=mybir.AluOpType.add)
                    if l < 2:
                        o2 = opool.tile([P, outw], b16, tag="o2")
                        nc.scalar.activation(
                            out=o2[:], in_=o[:],
                            func=mybir.ActivationFunctionType.Relu,
                        )
                        nc.sync.dma_start(out=dest[t * P : (t + 1) * P, :], in_=o2[:])
                    else:
                        nc.sync.dma_start(out=dest[:], in_=o[0:SEEDS_PER_CORE, :])
                if l < 2:
                    tc.strict_bb_all_engine_barrier()

    nc.compile()

    in_maps = []
    bf16 = _bf16()
    for c in range(NCORES):
        m = dict(xc=meta["xcs"][c])
        for l, plan in enumerate(plans):
            m[f"gidx{l}"] = _wrap_idx16(plan, c)
            m[f"sp{l}"] = np.ascontiguousarray(
                plan.wmat[c].astype(bf16).reshape(P, plan.n_cols_pad * P)
            )
            ws, wn, b = meta["weights"][l]
            m[f"ws{l}"] = np.ascontiguousarray(ws.astype(bf16))
            m[f"wn{l}"] = np.ascontiguousarray(wn.astype(bf16))
            m[f"bias{l}"] = np.broadcast_to(b[None, :], (P, OUTW[l])).copy()
        in_maps.append(m)

    res = run_bass_kernel_spmd(
        nc, in_maps, core_ids=list(range(NCORES)), trace=trace
    )
    return [res.results[c]["out"] for c in range(NCORES)], res


def kernel(**inputs) -> np.ndarray:
    meta = build_host(inputs)
    outs, _ = run_device(meta)
    return np.concatenate(outs, axis=0)


# revision 14
# speedup vs baseline: 1.0353x; 1.0353x over previous
"""DistSAGE 3-layer GraphSAGE forward on 8 TRN2 NeuronCores (Bass/Tile).

Strategy (graph/data parallel, per the DistSAGE recipe):
  - Partition the 512 seed nodes across 8 cores (64 each); build per-core
    dependency-driven blocks on the host (standard DGL block construction,
    pure index math): layer-2 dsts = seeds, layer-1 dsts = seeds + their
    layer-2 sources, layer-0 dsts = layer-1 dsts + their layer-1 sources.
    No inter-core communication; weights replicated.
  - Row-shard the feature table: each core gets a compact bf16 table with
    only the x rows its block touches, laid out in per-dst-tile "bands" so
    each dma_gather call addresses rows with int16 indices relative to a
    32768-row window (the gather ucode's index width).
  - Per 128-dst tile on device: dma_gather the tile's unique source rows
    (128/chunk, up to 4096/call), accumulate meanT[f,d] += msgs.T @ S' on
    the TensorEngine in PSUM, where S'[slot,d] = multiplicity(slot->d)/deg[d]
    is a host-precomputed bf16 matrix DMA'd on the (otherwise idle) HWDGE
    line.  A per-tile "self chunk" whose S' is the identity yields h_dstT
    the same way (doubling as the transpose).  Then Y[d,:] =
    meanT.T@W_neigh + h_dstT.T@W_self (4 PSUM-accumulated bf16 matmuls),
    + bias, ReLU, DMA the bf16 tile to DRAM for the next layer's gather.
"""

import heapq

import numpy as np

P = 128
NCORES = 8
NUM_DST = (61952, 5632, 512)
FEAT = 256
OUTW = (256, 256, 19)
SEEDS_PER_CORE = NUM_DST[2] // NCORES  # 64
WINDOW = 32768
NI_L = (2048, 2048, 1024)  # dma_gather indices per call, per layer


def _bf16():
    import ml_dtypes

    return ml_dtypes.bfloat16


# ---------------------------------------------------------------------------
# Host-side block construction
# ---------------------------------------------------------------------------


def _balance(ids, deg, n_buckets):
    """LPT bin-packing: reorder ids so consecutive 128-groups have ~equal
    total degree. len(ids) must be >= n_buckets * 128 is not required; only
    full groups are balanced."""
    if n_buckets <= 1 or len(ids) < n_buckets * P:
        return ids
    order = np.argsort(-deg[ids], kind="stable")
    heap = [(0.0, b, 0) for b in range(n_buckets)]
    heapq.heapify(heap)
    buckets = [[] for _ in range(n_buckets)]
    for i in order:
        load, b, cnt = heapq.heappop(heap)
        buckets[b].append(ids[i])
        cnt += 1
        if cnt < P:
            heapq.heappush(heap, (load + deg[ids[i]], b, cnt))
    return np.concatenate([np.asarray(b, dtype=ids.dtype) for b in buckets])


def _block_for_core(c, esrc0, edst0, esrc1, edst1, esrc2, edst2, deg0, deg1, deg2):
    seeds = np.arange(c * SEEDS_PER_CORE, (c + 1) * SEEDS_PER_CORE, dtype=np.int64)

    sel2 = (edst2 >= seeds[0]) & (edst2 < seeds[0] + SEEDS_PER_CORE)
    es2, ed2g = esrc2[sel2], edst2[sel2]
    l1_extra = np.setdiff1d(np.unique(es2), seeds)
    nfull = (len(l1_extra) // P) * P
    if nfull >= P:
        l1_extra = np.concatenate(
            [_balance(l1_extra[:nfull], deg1, nfull // P), l1_extra[nfull:]]
        )
    l1_out = np.concatenate([seeds, l1_extra])
    n1 = len(l1_out)

    pos1 = np.full(NUM_DST[1], -1, np.int32)
    pos1[l1_out] = np.arange(n1, dtype=np.int32)
    sel1 = pos1[edst1] >= 0
    es1, ed1g = esrc1[sel1], edst1[sel1]
    ed1 = pos1[ed1g].astype(np.int64)
    inv1 = (1.0 / np.maximum(deg1[ed1g], 1.0)).astype(np.float32)
    l0_extra = np.setdiff1d(np.unique(es1), l1_out)
    nfull = (len(l0_extra) // P) * P
    if nfull >= P:
        l0_extra = np.concatenate(
            [_balance(l0_extra[:nfull], deg0, nfull // P), l0_extra[nfull:]]
        )
    l0_out = np.concatenate([l1_out, l0_extra])
    n0 = len(l0_out)

    pos0 = np.full(NUM_DST[0], -1, np.int32)
    pos0[l0_out] = np.arange(n0, dtype=np.int32)
    sel0 = pos0[edst0] >= 0
    es0, ed0g = esrc0[sel0], edst0[sel0]
    ed0 = pos0[ed0g].astype(np.int64)
    inv0 = (1.0 / np.maximum(deg0[ed0g], 1.0)).astype(np.float32)

    ed2 = pos1[ed2g].astype(np.int64)
    inv2 = (1.0 / np.maximum(deg2[ed2g], 1.0)).astype(np.float32)
    es2l = pos1[es2].astype(np.int64)
    es1l = pos0[es1].astype(np.int64)

    return dict(
        l1_out=l1_out, l0_out=l0_out, n1=n1, n0=n0,
        e0=(es0.astype(np.int64), ed0, inv0),
        e1=(es1l, ed1, inv1),
        e2=(es2l, ed2, inv2),
    )


def _group_edges_by_tile(es, ed, inv, n_tiles):
    """Per dst-tile: dedup sources and build the dense S' payload.

    Returns per-tile (unique_srcs sorted, W [n_u, 128] f32) where
    W[slot, d] = sum of invdeg over edges (src_slot -> tile_base + d)."""
    tile = ed // P
    order = np.argsort(tile, kind="stable")
    es, ed, inv, tile = es[order], ed[order], inv[order], tile[order]
    starts = np.searchsorted(tile, np.arange(n_tiles))
    ends = np.searchsorted(tile, np.arange(n_tiles) + 1)
    out = []
    for t in range(n_tiles):
        s, e = starts[t], ends[t]
        u, ii = np.unique(es[s:e], return_inverse=True)
        W = np.zeros((len(u), P), np.float32)
        np.add.at(W, (ii, ed[s:e] - t * P), inv[s:e])
        out.append((u, W))
    return out


class LayerPlan:
    def __init__(self, n_tiles, chunks_per_tile, ni):
        self.ni = ni
        self.cpc = ni // P
        self.n_tiles = n_tiles
        self.chunks_per_tile = chunks_per_tile
        self.col_self = []
        self.col_edges = []
        col = 0
        for t in range(n_tiles):
            self.col_self.append(col)
            col += 1
            self.col_edges.append(list(range(col, col + chunks_per_tile[t])))
            col += chunks_per_tile[t]
        self.n_cols = col
        self.n_calls = -(-col // self.cpc)
        self.n_cols_pad = self.n_calls * self.cpc
        self.gidx = []  # [NCORES][128, n_cols_pad] int64 table rows
        self.wmat = []  # [NCORES][128, n_cols_pad, 128] f32 S' content
        self.call_base = None  # [n_calls] int64, uniform across cores


def _plan_layer(per_core_tiles, n_tiles, ni):
    chunks = [
        max(1, max(-(-len(per_core_tiles[c][t][0]) // P) for c in range(NCORES)))
        for t in range(n_tiles)
    ]
    return LayerPlan(n_tiles, chunks, ni)


def _fill_plan(plan, per_core_tiles, self_rows, pad_row):
    for c in range(NCORES):
        gidx = np.zeros((P, plan.n_cols_pad), np.int64)
        wmat = np.zeros((P, plan.n_cols_pad, P), np.float32)
        eye = np.eye(P, dtype=np.float32)
        for t in range(plan.n_tiles):
            gidx[:, plan.col_self[t]] = self_rows[c][t]
            wmat[:, plan.col_self[t], :] = eye
            u, W = per_core_tiles[c][t]
            n = len(u)
            cols = plan.col_edges[t]
            bi = np.full(len(cols) * P, pad_row[c][t], np.int64)
            bw = np.zeros((len(cols) * P, P), np.float32)
            bi[:n] = u
            bw[:n] = W
            for j, col in enumerate(cols):
                gidx[:, col] = bi[j * P : (j + 1) * P]
                wmat[:, col, :] = bw[j * P : (j + 1) * P]
        if plan.n_cols_pad > plan.n_cols:
            gidx[:, plan.n_cols :] = pad_row[c][plan.n_tiles - 1]
        plan.gidx.append(gidx)
        plan.wmat.append(wmat)


def build_host(inputs):
    esrc0 = np.asarray(inputs["esrc0"]).astype(np.int64)
    edst0 = np.asarray(inputs["edst0"]).astype(np.int64)
    esrc1 = np.asarray(inputs["esrc1"]).astype(np.int64)
    edst1 = np.asarray(inputs["edst1"]).astype(np.int64)
    esrc2 = np.asarray(inputs["esrc2"]).astype(np.int64)
    edst2 = np.asarray(inputs["edst2"]).astype(np.int64)
    x = np.asarray(inputs["x"], dtype=np.float32)

    deg0 = np.bincount(edst0, minlength=NUM_DST[0]).astype(np.float32)
    deg1 = np.bincount(edst1, minlength=NUM_DST[1]).astype(np.float32)
    deg2 = np.bincount(edst2, minlength=NUM_DST[2]).astype(np.float32)

    blocks = [
        _block_for_core(c, esrc0, edst0, esrc1, edst1, esrc2, edst2, deg0, deg1, deg2)
        for c in range(NCORES)
    ]

    n0_pad = max(-(-b["n0"] // P) for b in blocks) * P
    n1_pad = max(-(-b["n1"] // P) for b in blocks) * P
    T0, T1, T2 = n0_pad // P, n1_pad // P, 1

    tiles0 = [_group_edges_by_tile(*b["e0"], T0) for b in blocks]
    tiles1 = [_group_edges_by_tile(*b["e1"], T1) for b in blocks]
    tiles2 = [_group_edges_by_tile(*b["e2"], T2) for b in blocks]

    plan0 = _plan_layer(tiles0, T0, NI_L[0])
    plan1 = _plan_layer(tiles1, T1, NI_L[1])
    plan2 = _plan_layer(tiles2, T2, NI_L[2])

    l0_padded = []
    for b in blocks:
        v = np.zeros(T0 * P, np.int64)
        v[: b["n0"]] = b["l0_out"]
        v[b["n0"] :] = b["l0_out"][0]
        l0_padded.append(v)
    self0 = [
        [l0_padded[c][t * P : (t + 1) * P] for t in range(T0)] for c in range(NCORES)
    ]
    pad0 = [[l0_padded[c][t * P] for t in range(T0)] for c in range(NCORES)]
    _fill_plan(plan0, tiles0, self0, pad0)

    ar = np.arange(P, dtype=np.int64)
    selfL = lambda T: [[t * P + ar for t in range(T)] for _ in range(NCORES)]
    padL = lambda T: [[t * P for t in range(T)] for _ in range(NCORES)]
    _fill_plan(plan1, tiles1, selfL(T1), padL(T1))
    _fill_plan(plan2, tiles2, selfL(T2), padL(T2))

    # ---- layer 0: banded compact x table (uniform band offsets) ----
    bands = []
    for c in range(NCORES):
        bands.append(
            [
                np.unique(plan0.gidx[c][:, [plan0.col_self[t]] + plan0.col_edges[t]])
                for t in range(T0)
            ]
        )
    band_size = np.array(
        [max(len(bands[c][t]) for c in range(NCORES)) for t in range(T0)], np.int64
    )
    band_start = np.concatenate([[0], np.cumsum(band_size)])

    for c in range(NCORES):
        gidx, new = plan0.gidx[c], np.zeros_like(plan0.gidx[c])
        for t in range(T0):
            band = bands[c][t]
            cols = [plan0.col_self[t]] + plan0.col_edges[t]
            sl = gidx[:, cols]
            loc = np.searchsorted(band, sl)
            assert (band[loc] == sl).all()
            new[:, cols] = band_start[t] + loc
        if plan0.n_cols_pad > plan0.n_cols:
            band = bands[c][T0 - 1]
            sl = gidx[:, plan0.n_cols :]
            new[:, plan0.n_cols :] = band_start[T0 - 1] + np.searchsorted(band, sl)
        plan0.gidx[c] = new

    col_tile = np.zeros(plan0.n_cols_pad, np.int64)
    for t in range(T0):
        for col in [plan0.col_self[t]] + plan0.col_edges[t]:
            col_tile[col] = t
    col_tile[plan0.n_cols :] = T0 - 1
    cpc0 = plan0.cpc
    plan0.call_base = np.array(
        [band_start[col_tile[k * cpc0]] for k in range(plan0.n_calls)], np.int64
    )
    for c in range(NCORES):
        for k in range(plan0.n_calls):
            sl = plan0.gidx[c][:, k * cpc0 : (k + 1) * cpc0]
            assert sl.min() >= plan0.call_base[k], (c, k)
            assert sl.max() < plan0.call_base[k] + WINDOW, (c, k)
    plan1.call_base = np.zeros(plan1.n_calls, np.int64)
    plan2.call_base = np.zeros(plan2.n_calls, np.int64)
    assert n0_pad <= WINDOW and n1_pad <= WINDOW

    bf16 = _bf16()
    x16 = x.astype(bf16)
    xc_len_pad = -(-int(band_start[T0]) // P) * P
    xcs = []
    for c in range(NCORES):
        t = np.zeros((xc_len_pad, FEAT), bf16)
        for tt in range(T0):
            b = bands[c][tt]
            t[band_start[tt] : band_start[tt] + len(b)] = x16[b]
        xcs.append(t)

    return dict(
        plans=(plan0, plan1, plan2),
        T=(T0, T1, T2),
        n0_pad=n0_pad,
        n1_pad=n1_pad,
        xc_len_pad=xc_len_pad,
        xcs=xcs,
        blocks=blocks,
        weights=tuple(
            (
                np.asarray(inputs[f"W_self{l}"], np.float32),
                np.asarray(inputs[f"W_neigh{l}"], np.float32),
                np.asarray(inputs[f"b{l}"], np.float32),
            )
            for l in range(3)
        ),
    )


# ---------------------------------------------------------------------------
# Numpy simulation of the device kernel (validation aid; fp32 stand-in)
# ---------------------------------------------------------------------------


def simulate_core(meta, c):
    table = meta["xcs"][c].astype(np.float32)
    for l, plan in enumerate(meta["plans"]):
        ws, wn, b = meta["weights"][l]
        out = np.zeros((plan.n_tiles * P, OUTW[l]), np.float32)
        for t in range(plan.n_tiles):
            hd = table[plan.gidx[c][:, plan.col_self[t]]]
            aggT = np.zeros((FEAT, P), np.float32)
            for col in plan.col_edges[t]:
                msgs = table[plan.gidx[c][:, col]]
                aggT += msgs.T @ plan.wmat[c][:, col, :]
            y = hd @ ws + aggT.T @ wn + b
            if l < 2:
                y = np.maximum(y, 0.0)
            out[t * P : (t + 1) * P] = y
        table = out
    return table[:SEEDS_PER_CORE]


# ---------------------------------------------------------------------------
# Device kernel
# ---------------------------------------------------------------------------


def _wrap_idx16(plan, c):
    """Relative rows -> dma_gather idx layout [128, n_calls*ni/16] int16
    (16-partition wrap, replicated x8)."""
    ni, cpc = plan.ni, plan.cpc
    rel = plan.gidx[c] - np.repeat(plan.call_base, cpc)[None, :]
    n_calls = plan.n_calls
    out = np.zeros((P, n_calls * ni // 16), np.int16)
    for k in range(n_calls):
        flat = rel[:, k * cpc : (k + 1) * cpc].T.reshape(-1)  # i = j*128+p
        w = flat.reshape(ni // 16, 16).T.astype(np.int16)
        out[:16, k * (ni // 16) : (k + 1) * (ni // 16)] = w
    for rep in range(1, 8):
        out[rep * 16 : (rep + 1) * 16] = out[:16]
    return out


def run_device(meta, trace=False):
    import concourse.bacc as bacc
    import concourse.tile as tile
    import concourse.mybir as mybir
    from concourse.bass_utils import run_bass_kernel_spmd

    plans = meta["plans"]
    f32 = mybir.dt.float32
    b16 = mybir.dt.bfloat16

    nc = bacc.Bacc("TRN2", target_bir_lowering=False, debug=False, num_devices=NCORES)

    xc = nc.dram_tensor("xc", [meta["xc_len_pad"], FEAT], b16, kind="ExternalInput")
    h1buf = nc.dram_tensor("h1buf", [meta["n0_pad"], FEAT], b16)
    h2buf = nc.dram_tensor("h2buf", [meta["n1_pad"], FEAT], b16)
    out_d = nc.dram_tensor("out", [SEEDS_PER_CORE, OUTW[2]], f32, kind="ExternalOutput")

    idx_d, sp_d, w_d = [], [], []
    for l, plan in enumerate(plans):
        idx_d.append(
            nc.dram_tensor(f"gidx{l}", [P, plan.n_calls * plan.ni // 16],
                           mybir.dt.int16, kind="ExternalInput")
        )
        sp_d.append(
            nc.dram_tensor(f"sp{l}", [P, plan.n_cols_pad * P], b16,
                           kind="ExternalInput")
        )
        w_d.append(
            (
                nc.dram_tensor(f"ws{l}", [FEAT, OUTW[l]], b16, kind="ExternalInput"),
                nc.dram_tensor(f"wn{l}", [FEAT, OUTW[l]], b16, kind="ExternalInput"),
                nc.dram_tensor(f"bias{l}", [P, OUTW[l]], f32, kind="ExternalInput"),
            )
        )

    tables = [xc, h1buf, h2buf]
    dests = [h1buf, h2buf, out_d]

    with tile.TileContext(nc) as tc:
        with (
            tc.tile_pool(name="const", bufs=1) as cpool,
            tc.tile_pool(name="msgs", bufs=5) as mpool,
            tc.tile_pool(name="sel", bufs=5) as spool,
            tc.tile_pool(name="acc", bufs=2) as apool,
            tc.tile_pool(name="outp", bufs=3) as opool,
            tc.tile_pool(name="pagg", bufs=1, space="PSUM") as pa,
            tc.tile_pool(name="py", bufs=2, space="PSUM") as pypool,
        ):
            # preload every layer's constants up front (HWDGE line is idle
            # at kernel start; keeps layer transitions seamless)
            idx_ts, ws_ts, wn_ts, bias_ts = [], [], [], []
            for l, plan in enumerate(plans):
                outw = OUTW[l]
                idx_t = cpool.tile(list(idx_d[l].shape), mybir.dt.int16, tag=f"idx{l}")
                nc.sync.dma_start(out=idx_t[:], in_=idx_d[l][:])
                idx_ts.append(idx_t)
                wst, wnt = [], []
                for k in range(2):
                    w = cpool.tile([P, outw], b16, tag=f"ws{l}_{k}")
                    nc.sync.dma_start(out=w[:], in_=w_d[l][0][k * P : (k + 1) * P, :])
                    wst.append(w)
                    w = cpool.tile([P, outw], b16, tag=f"wn{l}_{k}")
                    nc.sync.dma_start(out=w[:], in_=w_d[l][1][k * P : (k + 1) * P, :])
                    wnt.append(w)
                ws_ts.append(wst)
                wn_ts.append(wnt)
                bias_t = cpool.tile([P, outw], f32, tag=f"bias{l}")
                nc.sync.dma_start(out=bias_t[:], in_=w_d[l][2][:])
                bias_ts.append(bias_t)

            for l, plan in enumerate(plans):
                outw = OUTW[l]
                table, dest = tables[l], dests[l]
                ws_t, wn_t, bias_t, idx_t = ws_ts[l], wn_ts[l], bias_ts[l], idx_ts[l]
                ni, cpc = plan.ni, plan.cpc

                call_tiles, sp_tiles = [], []
                for k in range(plan.n_calls):
                    mt = mpool.tile([P, cpc * FEAT], b16, tag="msgs")
                    base = int(plan.call_base[k])
                    hi = min(base + WINDOW, table.shape[0])
                    nc.gpsimd.dma_gather(
                        out_ap=mt[:, : cpc * FEAT].rearrange(
                            "p (g d) -> p g d", g=cpc
                        ),
                        in_ap=table[base:hi, :],
                        idxs_ap=idx_t[:, k * (ni // 16) : (k + 1) * (ni // 16)],
                        num_idxs=ni,
                        num_idxs_reg=ni,
                        elem_size=FEAT,
                        single_packet=False,
                    )
                    call_tiles.append(mt)
                    st = spool.tile([P, cpc * P], b16, tag="sp")
                    nc.sync.dma_start(
                        out=st[:, : cpc * P],
                        in_=sp_d[l][:, k * cpc * P : (k + 1) * cpc * P],
                    )
                    sp_tiles.append(st)

                def msg_slice(col, f0, f1):
                    k, j = divmod(col, cpc)
                    return call_tiles[k][:, j * FEAT + f0 : j * FEAT + f1]

                def sp_slice(col):
                    k, j = divmod(col, cpc)
                    return sp_tiles[k][:, j * P : (j + 1) * P]

                for t in range(plan.n_tiles):
                    cs = plan.col_self[t]
                    ph0 = pa.tile([P, P], f32, tag="ph0")
                    ph1 = pa.tile([P, P], f32, tag="ph1")
                    nc.tensor.matmul(ph0[:], lhsT=msg_slice(cs, 0, P),
                                     rhs=sp_slice(cs), start=True, stop=True)
                    nc.tensor.matmul(ph1[:], lhsT=msg_slice(cs, P, 2 * P),
                                     rhs=sp_slice(cs), start=True, stop=True)
                    pa0 = pa.tile([P, P], f32, tag="pa0")
                    pa1 = pa.tile([P, P], f32, tag="pa1")
                    cols = plan.col_edges[t]
                    for i, col in enumerate(cols):
                        st, sp = (i == 0), (i == len(cols) - 1)
                        nc.tensor.matmul(pa0[:], lhsT=msg_slice(col, 0, P),
                                         rhs=sp_slice(col), start=st, stop=sp)
                        nc.tensor.matmul(pa1[:], lhsT=msg_slice(col, P, 2 * P),
                                         rhs=sp_slice(col), start=st, stop=sp)
                    a0 = apool.tile([P, P], b16, tag="a0")
                    nc.vector.tensor_copy(out=a0[:], in_=pa0[:])
                    a1 = apool.tile([P, P], b16, tag="a1")
                    nc.vector.tensor_copy(out=a1[:], in_=pa1[:])
                    h0 = apool.tile([P, P], b16, tag="h0")
                    nc.vector.tensor_copy(out=h0[:], in_=ph0[:])
                    h1 = apool.tile([P, P], b16, tag="h1")
                    nc.vector.tensor_copy(out=h1[:], in_=ph1[:])
                    y = pypool.tile([P, outw], f32, tag="y")
                    nc.tensor.matmul(y[:], lhsT=a0[:], rhs=wn_t[0][:],
                                     start=True, stop=False)
                    nc.tensor.matmul(y[:], lhsT=a1[:], rhs=wn_t[1][:],
                                     start=False, stop=False)
                    nc.tensor.matmul(y[:], lhsT=h0[:], rhs=ws_t[0][:],
                                     start=False, stop=False)
                    nc.tensor.matmul(y[:], lhsT=h1[:], rhs=ws_t[1][:],
                                     start=False, stop=True)
                    o = opool.tile([P, outw], f32, tag="o")
                    nc.vector.tensor_tensor(out=o[:], in0=y[:], in1=bias_t[:],
                                            op=mybir.AluOpType.add)
                    if l < 2:
                        o2 = opool.tile([P, outw], b16, tag="o2")
                        nc.scalar.activation(
                            out=o2[:], in_=o[:],
                            func=mybir.ActivationFunctionType.Relu,
                        )
                        nc.sync.dma_start(out=dest[t * P : (t + 1) * P, :], in_=o2[:])
                    else:
                        nc.sync.dma_start(out=dest[:], in_=o[0:SEEDS_PER_CORE, :])
                if l < 2:
                    tc.strict_bb_all_engine_barrier()

    nc.compile()

    in_maps = []
    bf16 = _bf16()
    for c in range(NCORES):
        m = dict(xc=meta["xcs"][c])
        for l, plan in enumerate(plans):
            m[f"gidx{l}"] = _wrap_idx16(plan, c)
            m[f"sp{l}"] = np.ascontiguousarray(
                plan.wmat[c].astype(bf16).reshape(P, plan.n_cols_pad * P)
            )
            ws, wn, b = meta["weights"][l]
            m[f"ws{l}"] = np.ascontiguousarray(ws.astype(bf16))
            m[f"wn{l}"] = np.ascontiguousarray(wn.astype(bf16))
            m[f"bias{l}"] = np.broadcast_to(b[None, :], (P, OUTW[l])).copy()
        in_maps.append(m)

    res = run_bass_kernel_spmd(
        nc, in_maps, core_ids=list(range(NCORES)), trace=trace
    )
    return [res.results[c]["out"] for c in range(NCORES)], res


def kernel(**inputs) -> np.ndarray:
    meta = build_host(inputs)
    outs, _ = run_device(meta)
    return np.concatenate(outs, axis=0)


# revision 16
# speedup vs baseline: 1.1684x; 1.1285x over previous
"""DistSAGE 3-layer GraphSAGE forward on 8 TRN2 NeuronCores (Bass/Tile).

Strategy (graph/data parallel, per the DistSAGE recipe):
  - Partition the 512 seed nodes across 8 cores (64 each); build per-core
    dependency-driven blocks on the host (standard DGL block construction,
    pure index math): layer-2 dsts = seeds, layer-1 dsts = seeds + their
    layer-2 sources, layer-0 dsts = layer-1 dsts + their layer-1 sources.
    No inter-core communication; weights replicated.
  - Row-shard the feature table: each core gets a compact bf16 table with
    only the x rows its block touches, laid out in per-dst-tile "bands" so
    each dma_gather call addresses rows with int16 indices relative to a
    32768-row window (the gather ucode's index width).
  - Per 128-dst tile on device: dma_gather the tile's unique source rows
    (128/chunk, up to 4096/call), accumulate meanT[f,d] += msgs.T @ S' on
    the TensorEngine in PSUM, where S'[slot,d] = multiplicity(slot->d)/deg[d]
    is a host-precomputed bf16 matrix DMA'd on the (otherwise idle) HWDGE
    line.  A per-tile "self chunk" whose S' is the identity yields h_dstT
    the same way (doubling as the transpose).  Then Y[d,:] =
    meanT.T@W_neigh + h_dstT.T@W_self (4 PSUM-accumulated bf16 matmuls),
    + bias, ReLU, DMA the bf16 tile to DRAM for the next layer's gather.
"""

import heapq

import numpy as np

P = 128
NCORES = 8
NUM_DST = (61952, 5632, 512)
FEAT = 256
OUTW = (256, 256, 19)
SEEDS_PER_CORE = NUM_DST[2] // NCORES  # 64
WINDOW = 32768
NI_L = (2048, 2048, 1024)  # dma_gather indices per call, per layer


def _bf16():
    import ml_dtypes

    return ml_dtypes.bfloat16


# ---------------------------------------------------------------------------
# Host-side block construction
# ---------------------------------------------------------------------------


def _balance(ids, deg, n_buckets):
    """LPT bin-packing: reorder ids so consecutive 128-groups have ~equal
    total degree. len(ids) must be >= n_buckets * 128 is not required; only
    full groups are balanced."""
    if n_buckets <= 1 or len(ids) < n_buckets * P:
        return ids
    order = np.argsort(-deg[ids], kind="stable")
    heap = [(0.0, b, 0) for b in range(n_buckets)]
    heapq.heapify(heap)
    buckets = [[] for _ in range(n_buckets)]
    for i in order:
        load, b, cnt = heapq.heappop(heap)
        buckets[b].append(ids[i])
        cnt += 1
        if cnt < P:
            heapq.heappush(heap, (load + deg[ids[i]], b, cnt))
    return np.concatenate([np.asarray(b, dtype=ids.dtype) for b in buckets])


def _block_for_core(c, esrc0, edst0, esrc1, edst1, esrc2, edst2, deg0, deg1, deg2):
    seeds = np.arange(c * SEEDS_PER_CORE, (c + 1) * SEEDS_PER_CORE, dtype=np.int64)

    sel2 = (edst2 >= seeds[0]) & (edst2 < seeds[0] + SEEDS_PER_CORE)
    es2, ed2g = esrc2[sel2], edst2[sel2]
    l1_extra = np.setdiff1d(np.unique(es2), seeds)
    nfull = (len(l1_extra) // P) * P
    if nfull >= P:
        l1_extra = np.concatenate(
            [_balance(l1_extra[:nfull], deg1, nfull // P), l1_extra[nfull:]]
        )
    l1_out = np.concatenate([seeds, l1_extra])
    n1 = len(l1_out)

    pos1 = np.full(NUM_DST[1], -1, np.int32)
    pos1[l1_out] = np.arange(n1, dtype=np.int32)
    sel1 = pos1[edst1] >= 0
    es1, ed1g = esrc1[sel1], edst1[sel1]
    ed1 = pos1[ed1g].astype(np.int64)
    inv1 = (1.0 / np.maximum(deg1[ed1g], 1.0)).astype(np.float32)
    l0_extra = np.setdiff1d(np.unique(es1), l1_out)
    nfull = (len(l0_extra) // P) * P
    if nfull >= P:
        l0_extra = np.concatenate(
            [_balance(l0_extra[:nfull], deg0, nfull // P), l0_extra[nfull:]]
        )
    l0_out = np.concatenate([l1_out, l0_extra])
    n0 = len(l0_out)

    pos0 = np.full(NUM_DST[0], -1, np.int32)
    pos0[l0_out] = np.arange(n0, dtype=np.int32)
    sel0 = pos0[edst0] >= 0
    es0, ed0g = esrc0[sel0], edst0[sel0]
    ed0 = pos0[ed0g].astype(np.int64)
    inv0 = (1.0 / np.maximum(deg0[ed0g], 1.0)).astype(np.float32)

    ed2 = pos1[ed2g].astype(np.int64)
    inv2 = (1.0 / np.maximum(deg2[ed2g], 1.0)).astype(np.float32)
    es2l = pos1[es2].astype(np.int64)
    es1l = pos0[es1].astype(np.int64)

    return dict(
        l1_out=l1_out, l0_out=l0_out, n1=n1, n0=n0,
        e0=(es0.astype(np.int64), ed0, inv0),
        e1=(es1l, ed1, inv1),
        e2=(es2l, ed2, inv2),
    )


def _group_edges_by_tile(es, ed, inv, n_tiles):
    """Per dst-tile: dedup sources and build the dense S' payload.

    Returns per-tile (unique_srcs sorted, W [n_u, 128] f32) where
    W[slot, d] = sum of invdeg over edges (src_slot -> tile_base + d)."""
    tile = ed // P
    order = np.argsort(tile, kind="stable")
    es, ed, inv, tile = es[order], ed[order], inv[order], tile[order]
    starts = np.searchsorted(tile, np.arange(n_tiles))
    ends = np.searchsorted(tile, np.arange(n_tiles) + 1)
    out = []
    for t in range(n_tiles):
        s, e = starts[t], ends[t]
        u, ii = np.unique(es[s:e], return_inverse=True)
        W = np.zeros((len(u), P), np.float32)
        np.add.at(W, (ii, ed[s:e] - t * P), inv[s:e])
        out.append((u, W))
    return out


class LayerPlan:
    def __init__(self, n_tiles, chunks_per_tile, ni):
        self.ni = ni
        self.cpc = ni // P
        self.n_tiles = n_tiles
        self.chunks_per_tile = chunks_per_tile
        self.col_edges = []
        col = 0
        for t in range(n_tiles):
            self.col_edges.append(list(range(col, col + chunks_per_tile[t])))
            col += chunks_per_tile[t]
        self.n_cols = col
        self.n_calls = -(-col // self.cpc)
        self.n_cols_pad = self.n_calls * self.cpc
        self.gidx = []  # [NCORES][128, n_cols_pad] int64 table rows
        self.wmat = []  # [NCORES][128, n_cols_pad, 128] f32 S' content
        self.call_base = None  # [n_calls] int64, uniform across cores


def _plan_layer(per_core_tiles, n_tiles, ni):
    chunks = [
        max(1, max(-(-len(per_core_tiles[c][t][0]) // P) for c in range(NCORES)))
        for t in range(n_tiles)
    ]
    return LayerPlan(n_tiles, chunks, ni)


def _fill_plan(plan, per_core_tiles, pad_row):
    for c in range(NCORES):
        gidx = np.zeros((P, plan.n_cols_pad), np.int64)
        wmat = np.zeros((P, plan.n_cols_pad, P), np.float32)
        for t in range(plan.n_tiles):
            u, W = per_core_tiles[c][t]
            n = len(u)
            cols = plan.col_edges[t]
            bi = np.full(len(cols) * P, pad_row[c][t], np.int64)
            bw = np.zeros((len(cols) * P, P), np.float32)
            bi[:n] = u
            bw[:n] = W
            for j, col in enumerate(cols):
                gidx[:, col] = bi[j * P : (j + 1) * P]
                wmat[:, col, :] = bw[j * P : (j + 1) * P]
        if plan.n_cols_pad > plan.n_cols:
            gidx[:, plan.n_cols :] = pad_row[c][plan.n_tiles - 1]
        plan.gidx.append(gidx)
        plan.wmat.append(wmat)


def build_host(inputs):
    esrc0 = np.asarray(inputs["esrc0"]).astype(np.int64)
    edst0 = np.asarray(inputs["edst0"]).astype(np.int64)
    esrc1 = np.asarray(inputs["esrc1"]).astype(np.int64)
    edst1 = np.asarray(inputs["edst1"]).astype(np.int64)
    esrc2 = np.asarray(inputs["esrc2"]).astype(np.int64)
    edst2 = np.asarray(inputs["edst2"]).astype(np.int64)
    x = np.asarray(inputs["x"], dtype=np.float32)

    deg0 = np.bincount(edst0, minlength=NUM_DST[0]).astype(np.float32)
    deg1 = np.bincount(edst1, minlength=NUM_DST[1]).astype(np.float32)
    deg2 = np.bincount(edst2, minlength=NUM_DST[2]).astype(np.float32)

    blocks = [
        _block_for_core(c, esrc0, edst0, esrc1, edst1, esrc2, edst2, deg0, deg1, deg2)
        for c in range(NCORES)
    ]

    n0_pad = max(-(-b["n0"] // P) for b in blocks) * P
    n1_pad = max(-(-b["n1"] // P) for b in blocks) * P
    T0, T1, T2 = n0_pad // P, n1_pad // P, 1

    tiles0 = [_group_edges_by_tile(*b["e0"], T0) for b in blocks]
    tiles1 = [_group_edges_by_tile(*b["e1"], T1) for b in blocks]
    tiles2 = [_group_edges_by_tile(*b["e2"], T2) for b in blocks]

    plan0 = _plan_layer(tiles0, T0, NI_L[0])
    plan1 = _plan_layer(tiles1, T1, NI_L[1])
    plan2 = _plan_layer(tiles2, T2, NI_L[2])

    l0_padded = []
    for b in blocks:
        v = np.zeros(T0 * P, np.int64)
        v[: b["n0"]] = b["l0_out"]
        v[b["n0"] :] = b["l0_out"][0]
        l0_padded.append(v)
    pad0 = [[l0_padded[c][t * P] for t in range(T0)] for c in range(NCORES)]
    _fill_plan(plan0, tiles0, pad0)

    padL = lambda T: [[t * P for t in range(T)] for _ in range(NCORES)]
    _fill_plan(plan1, tiles1, padL(T1))
    _fill_plan(plan2, tiles2, padL(T2))

    # ---- layer 0: banded compact x table (uniform band offsets) ----
    # band t = [tile t's 128 dst rows in slot order | sorted unique srcs];
    # the dst part feeds a dense h_dst DMA, the src part the gathers.
    src_bands = []
    for c in range(NCORES):
        src_bands.append(
            [
                np.unique(plan0.gidx[c][:, plan0.col_edges[t]])
                for t in range(T0)
            ]
        )
    band_size = np.array(
        [P + max(len(src_bands[c][t]) for c in range(NCORES)) for t in range(T0)],
        np.int64,
    )
    band_start = np.concatenate([[0], np.cumsum(band_size)])

    for c in range(NCORES):
        gidx, new = plan0.gidx[c], np.zeros_like(plan0.gidx[c])
        for t in range(T0):
            band = src_bands[c][t]
            cols = plan0.col_edges[t]
            sl = gidx[:, cols]
            loc = np.searchsorted(band, sl)
            assert (band[loc] == sl).all()
            new[:, cols] = band_start[t] + P + loc
        if plan0.n_cols_pad > plan0.n_cols:
            band = src_bands[c][T0 - 1]
            sl = gidx[:, plan0.n_cols :]
            new[:, plan0.n_cols :] = (
                band_start[T0 - 1] + P + np.searchsorted(band, sl)
            )
        plan0.gidx[c] = new

    col_tile = np.zeros(plan0.n_cols_pad, np.int64)
    for t in range(T0):
        for col in plan0.col_edges[t]:
            col_tile[col] = t
    col_tile[plan0.n_cols :] = T0 - 1
    cpc0 = plan0.cpc
    plan0.call_base = np.array(
        [band_start[col_tile[k * cpc0]] for k in range(plan0.n_calls)], np.int64
    )
    for c in range(NCORES):
        for k in range(plan0.n_calls):
            sl = plan0.gidx[c][:, k * cpc0 : (k + 1) * cpc0]
            assert sl.min() >= plan0.call_base[k], (c, k)
            assert sl.max() < plan0.call_base[k] + WINDOW, (c, k)
    plan1.call_base = np.zeros(plan1.n_calls, np.int64)
    plan2.call_base = np.zeros(plan2.n_calls, np.int64)
    assert n0_pad <= WINDOW and n1_pad <= WINDOW

    bf16 = _bf16()
    x16 = x.astype(bf16)
    xc_len_pad = -(-int(band_start[T0]) // P) * P
    xcs = []
    for c in range(NCORES):
        t = np.zeros((xc_len_pad, FEAT), bf16)
        for tt in range(T0):
            o = band_start[tt]
            t[o : o + P] = x16[l0_padded[c][tt * P : (tt + 1) * P]]
            b = src_bands[c][tt]
            t[o + P : o + P + len(b)] = x16[b]
        xcs.append(t)

    return dict(
        plans=(plan0, plan1, plan2),
        band_start=band_start,
        T=(T0, T1, T2),
        n0_pad=n0_pad,
        n1_pad=n1_pad,
        xc_len_pad=xc_len_pad,
        xcs=xcs,
        blocks=blocks,
        weights=tuple(
            (
                np.asarray(inputs[f"W_self{l}"], np.float32),
                np.asarray(inputs[f"W_neigh{l}"], np.float32),
                np.asarray(inputs[f"b{l}"], np.float32),
            )
            for l in range(3)
        ),
    )


# ---------------------------------------------------------------------------
# Numpy simulation of the device kernel (validation aid; fp32 stand-in)
# ---------------------------------------------------------------------------


def simulate_core(meta, c):
    table = meta["xcs"][c].astype(np.float32)
    band_start = meta["band_start"]
    for l, plan in enumerate(meta["plans"]):
        ws, wn, b = meta["weights"][l]
        out = np.zeros((plan.n_tiles * P, OUTW[l]), np.float32)
        for t in range(plan.n_tiles):
            hd_off = band_start[t] if l == 0 else t * P
            hd = table[hd_off : hd_off + P]
            aggT = np.zeros((FEAT, P), np.float32)
            for col in plan.col_edges[t]:
                msgs = table[plan.gidx[c][:, col]]
                aggT += msgs.T @ plan.wmat[c][:, col, :]
            y = hd @ ws + aggT.T @ wn + b
            if l < 2:
                y = np.maximum(y, 0.0)
            out[t * P : (t + 1) * P] = y
        table = out
    return table[:SEEDS_PER_CORE]


# ---------------------------------------------------------------------------
# Device kernel
# ---------------------------------------------------------------------------


def _wrap_idx16(plan, c):
    """Relative rows -> dma_gather idx layout [128, n_calls*ni/16] int16
    (16-partition wrap, replicated x8)."""
    ni, cpc = plan.ni, plan.cpc
    rel = plan.gidx[c] - np.repeat(plan.call_base, cpc)[None, :]
    n_calls = plan.n_calls
    out = np.zeros((P, n_calls * ni // 16), np.int16)
    for k in range(n_calls):
        flat = rel[:, k * cpc : (k + 1) * cpc].T.reshape(-1)  # i = j*128+p
        w = flat.reshape(ni // 16, 16).T.astype(np.int16)
        out[:16, k * (ni // 16) : (k + 1) * (ni // 16)] = w
    for rep in range(1, 8):
        out[rep * 16 : (rep + 1) * 16] = out[:16]
    return out


def run_device(meta, trace=False):
    import concourse.bacc as bacc
    import concourse.tile as tile
    import concourse.mybir as mybir
    from concourse.bass_utils import run_bass_kernel_spmd

    plans = meta["plans"]
    f32 = mybir.dt.float32
    b16 = mybir.dt.bfloat16

    nc = bacc.Bacc("TRN2", target_bir_lowering=False, debug=False, num_devices=NCORES)

    xc = nc.dram_tensor("xc", [meta["xc_len_pad"], FEAT], b16, kind="ExternalInput")
    ident_d = nc.dram_tensor("ident", [P, P], b16, kind="ExternalInput")
    h1buf = nc.dram_tensor("h1buf", [meta["n0_pad"], FEAT], b16)
    h2buf = nc.dram_tensor("h2buf", [meta["n1_pad"], FEAT], b16)
    out_d = nc.dram_tensor("out", [SEEDS_PER_CORE, OUTW[2]], f32, kind="ExternalOutput")

    idx_d, sp_d, w_d = [], [], []
    for l, plan in enumerate(plans):
        idx_d.append(
            nc.dram_tensor(f"gidx{l}", [P, plan.n_calls * plan.ni // 16],
                           mybir.dt.int16, kind="ExternalInput")
        )
        sp_d.append(
            nc.dram_tensor(f"sp{l}", [P, plan.n_cols_pad * P], b16,
                           kind="ExternalInput")
        )
        w_d.append(
            (
                nc.dram_tensor(f"ws{l}", [FEAT, OUTW[l]], b16, kind="ExternalInput"),
                nc.dram_tensor(f"wn{l}", [FEAT, OUTW[l]], b16, kind="ExternalInput"),
                nc.dram_tensor(f"bias{l}", [P, OUTW[l]], f32, kind="ExternalInput"),
            )
        )

    tables = [xc, h1buf, h2buf]
    dests = [h1buf, h2buf, out_d]

    with tile.TileContext(nc) as tc:
        with (
            tc.tile_pool(name="const", bufs=1) as cpool,
            tc.tile_pool(name="msgs", bufs=5) as mpool,
            tc.tile_pool(name="sel", bufs=5) as spool,
            tc.tile_pool(name="acc", bufs=2) as apool,
            tc.tile_pool(name="outp", bufs=3) as opool,
            tc.tile_pool(name="pagg", bufs=1, space="PSUM") as pa,
            tc.tile_pool(name="py", bufs=2, space="PSUM") as pypool,
        ):
            # preload every layer's constants up front (HWDGE line is idle
            # at kernel start; keeps layer transitions seamless)
            ident_t = cpool.tile([P, P], b16, tag="ident")
            nc.sync.dma_start(out=ident_t[:], in_=ident_d[:])
            idx_ts, ws_ts, wn_ts, bias_ts = [], [], [], []
            for l, plan in enumerate(plans):
                outw = OUTW[l]
                idx_t = cpool.tile(list(idx_d[l].shape), mybir.dt.int16, tag=f"idx{l}")
                nc.sync.dma_start(out=idx_t[:], in_=idx_d[l][:])
                idx_ts.append(idx_t)
                wst, wnt = [], []
                for k in range(2):
                    w = cpool.tile([P, outw], b16, tag=f"ws{l}_{k}")
                    nc.sync.dma_start(out=w[:], in_=w_d[l][0][k * P : (k + 1) * P, :])
                    wst.append(w)
                    w = cpool.tile([P, outw], b16, tag=f"wn{l}_{k}")
                    nc.sync.dma_start(out=w[:], in_=w_d[l][1][k * P : (k + 1) * P, :])
                    wnt.append(w)
                ws_ts.append(wst)
                wn_ts.append(wnt)
                bias_t = cpool.tile([P, outw], f32, tag=f"bias{l}")
                nc.sync.dma_start(out=bias_t[:], in_=w_d[l][2][:])
                bias_ts.append(bias_t)

            for l, plan in enumerate(plans):
                outw = OUTW[l]
                table, dest = tables[l], dests[l]
                ws_t, wn_t, bias_t, idx_t = ws_ts[l], wn_ts[l], bias_ts[l], idx_ts[l]
                ni, cpc = plan.ni, plan.cpc

                call_tiles, sp_tiles = [], []
                for k in range(plan.n_calls):
                    mt = mpool.tile([P, cpc * FEAT], b16, tag="msgs")
                    base = int(plan.call_base[k])
                    hi = min(base + WINDOW, table.shape[0])
                    nc.gpsimd.dma_gather(
                        out_ap=mt[:, : cpc * FEAT].rearrange(
                            "p (g d) -> p g d", g=cpc
                        ),
                        in_ap=table[base:hi, :],
                        idxs_ap=idx_t[:, k * (ni // 16) : (k + 1) * (ni // 16)],
                        num_idxs=ni,
                        num_idxs_reg=ni,
                        elem_size=FEAT,
                        single_packet=False,
                    )
                    call_tiles.append(mt)
                    st = spool.tile([P, cpc * P], b16, tag="sp")
                    nc.sync.dma_start(
                        out=st[:, : cpc * P],
                        in_=sp_d[l][:, k * cpc * P : (k + 1) * cpc * P],
                    )
                    sp_tiles.append(st)

                def msg_slice(col, f0, f1):
                    k, j = divmod(col, cpc)
                    return call_tiles[k][:, j * FEAT + f0 : j * FEAT + f1]

                def sp_slice(col):
                    k, j = divmod(col, cpc)
                    return sp_tiles[k][:, j * P : (j + 1) * P]

                band_start = meta["band_start"]
                for t in range(plan.n_tiles):
                    hd_off = int(band_start[t]) if l == 0 else t * P
                    hd = opool.tile([P, FEAT], b16, tag="hd")
                    nc.sync.dma_start(out=hd[:], in_=table[hd_off : hd_off + P, :])
                    ph0 = pa.tile([P, P], f32, tag="ph0")
                    ph1 = pa.tile([P, P], f32, tag="ph1")
                    nc.tensor.matmul(ph0[:], lhsT=hd[:, 0:P], rhs=ident_t[:],
                                     start=True, stop=True)
                    nc.tensor.matmul(ph1[:], lhsT=hd[:, P : 2 * P], rhs=ident_t[:],
                                     start=True, stop=True)
                    pa0 = pa.tile([P, P], f32, tag="pa0")
                    pa1 = pa.tile([P, P], f32, tag="pa1")
                    cols = plan.col_edges[t]
                    for i, col in enumerate(cols):
                        st, sp = (i == 0), (i == len(cols) - 1)
                        nc.tensor.matmul(pa0[:], lhsT=msg_slice(col, 0, P),
                                         rhs=sp_slice(col), start=st, stop=sp)
                        nc.tensor.matmul(pa1[:], lhsT=msg_slice(col, P, 2 * P),
                                         rhs=sp_slice(col), start=st, stop=sp)
                    a0 = apool.tile([P, P], b16, tag="a0")
                    nc.vector.tensor_copy(out=a0[:], in_=pa0[:])
                    a1 = apool.tile([P, P], b16, tag="a1")
                    nc.vector.tensor_copy(out=a1[:], in_=pa1[:])
                    h0 = apool.tile([P, P], b16, tag="h0")
                    nc.vector.tensor_copy(out=h0[:], in_=ph0[:])
                    h1 = apool.tile([P, P], b16, tag="h1")
                    nc.vector.tensor_copy(out=h1[:], in_=ph1[:])
                    y = pypool.tile([P, outw], f32, tag="y")
                    nc.tensor.matmul(y[:], lhsT=a0[:], rhs=wn_t[0][:],
                                     start=True, stop=False)
                    nc.tensor.matmul(y[:], lhsT=a1[:], rhs=wn_t[1][:],
                                     start=False, stop=False)
                    nc.tensor.matmul(y[:], lhsT=h0[:], rhs=ws_t[0][:],
                                     start=False, stop=False)
                    nc.tensor.matmul(y[:], lhsT=h1[:], rhs=ws_t[1][:],
                                     start=False, stop=True)
                    o = opool.tile([P, outw], f32, tag="o")
                    nc.vector.tensor_tensor(out=o[:], in0=y[:], in1=bias_t[:],
                                            op=mybir.AluOpType.add)
                    if l < 2:
                        o2 = opool.tile([P, outw], b16, tag="o2")
                        nc.scalar.activation(
                            out=o2[:], in_=o[:],
                            func=mybir.ActivationFunctionType.Relu,
                        )
                        nc.sync.dma_start(out=dest[t * P : (t + 1) * P, :], in_=o2[:])
                    else:
                        nc.sync.dma_start(out=dest[:], in_=o[0:SEEDS_PER_CORE, :])

    nc.compile()

    in_maps = []
    bf16 = _bf16()
    eye16 = np.eye(P, dtype=bf16)
    for c in range(NCORES):
        m = dict(xc=meta["xcs"][c], ident=eye16)
        for l, plan in enumerate(plans):
            m[f"gidx{l}"] = _wrap_idx16(plan, c)
            m[f"sp{l}"] = np.ascontiguousarray(
                plan.wmat[c].astype(bf16).reshape(P, plan.n_cols_pad * P)
            )
            ws, wn, b = meta["weights"][l]
            m[f"ws{l}"] = np.ascontiguousarray(ws.astype(bf16))
            m[f"wn{l}"] = np.ascontiguousarray(wn.astype(bf16))
            m[f"bias{l}"] = np.broadcast_to(b[None, :], (P, OUTW[l])).copy()
        in_maps.append(m)

    res = run_bass_kernel_spmd(
        nc, in_maps, core_ids=list(range(NCORES)), trace=trace
    )
    return [res.results[c]["out"] for c in range(NCORES)], res


def kernel(**inputs) -> np.ndarray:
    meta = build_host(inputs)
    outs, _ = run_device(meta)
    return np.concatenate(outs, axis=0)


# revision 19
# speedup vs baseline: 1.2706x; 1.0875x over previous
"""DistSAGE 3-layer GraphSAGE forward on 8 TRN2 NeuronCores (Bass/Tile).

Strategy (graph/data parallel, per the DistSAGE recipe):
  - Partition the 512 seed nodes across 8 cores (64 each, LPT-balanced by
    an additive 2-hop cost estimate); build per-core dependency-driven
    blocks on the host (standard DGL block construction, pure index math).
    No inter-core communication; weights replicated.
  - Row-shard the feature table: each core gets a compact bf16 table with
    only the x rows its block touches, laid out in per-dst-tile "bands"
    ([128 h_dst rows | the tile's unique source rows]) so each dma_gather
    call addresses rows with int16 indices relative to a 32768-row window
    (the gather ucode's index width), and h_dst tiles load as dense DMAs.
  - The per-tile unique sources form one continuous slot stream gathered
    128/chunk, 2048/call.  Chunks need not align to tile boundaries: a
    boundary chunk contributes to both tiles via two host-baked masked S'
    columns.  meanT[f,d] += msgs.T @ S' accumulates on the TensorEngine in
    PSUM (S'[slot,d] = multiplicity(slot->d)/deg[d], bf16, DMA'd on the
    otherwise-idle HWDGE line); h_dstT comes from 2 identity matmuls of
    the dense h_dst tile.  Then Y[d,:] = meanT.T@W_neigh + h_dstT.T@W_self
    + 1s^T@bias (PSUM-accumulated bf16 matmuls), ReLU from PSUM, DMA the
    bf16 tile to DRAM for the next layer's gather.
"""

import heapq

import numpy as np

P = 128
NCORES = 8
NUM_DST = (61952, 5632, 512)
FEAT = 256
OUTW = (256, 256, 19)
SEEDS_PER_CORE = NUM_DST[2] // NCORES  # 64
WINDOW = 32768
NI_L = (2048, 2048, 1024)  # dma_gather indices per call, per layer


def _bf16():
    import ml_dtypes

    return ml_dtypes.bfloat16


# ---------------------------------------------------------------------------
# Host-side block construction
# ---------------------------------------------------------------------------


def _balance(ids, deg, n_buckets):
    """LPT bin-packing: reorder ids so consecutive 128-groups have ~equal
    total degree (only full 128-groups are balanced)."""
    if n_buckets <= 1 or len(ids) < n_buckets * P:
        return ids
    order = np.argsort(-deg[ids], kind="stable")
    heap = [(0.0, b, 0) for b in range(n_buckets)]
    heapq.heapify(heap)
    buckets = [[] for _ in range(n_buckets)]
    for i in order:
        load, b, cnt = heapq.heappop(heap)
        buckets[b].append(ids[i])
        cnt += 1
        if cnt < P:
            heapq.heappush(heap, (load + deg[ids[i]], b, cnt))
    return np.concatenate([np.asarray(b, dtype=ids.dtype) for b in buckets])


def _seed_partition(esrc0, edst0, esrc1, edst1, esrc2, edst2, deg0, deg1):
    """LPT-balance seeds across cores by an additive 2-hop cost estimate."""
    h = np.zeros(NUM_DST[1], np.float64)
    np.add.at(h, edst1, deg0[esrc1].astype(np.float64))
    cost = np.zeros(NUM_DST[2], np.float64)
    np.add.at(cost, edst2, h[esrc2] + deg1[esrc2].astype(np.float64))
    order = np.argsort(-cost, kind="stable")
    heap = [(0.0, cc, 0) for cc in range(NCORES)]
    heapq.heapify(heap)
    groups = [[] for _ in range(NCORES)]
    for s in order:
        load, cc, cnt = heapq.heappop(heap)
        groups[cc].append(s)
        cnt += 1
        if cnt < SEEDS_PER_CORE:
            heapq.heappush(heap, (load + cost[s], cc, cnt))
    return [np.array(g, dtype=np.int64) for g in groups]


def _block_for_core(seeds, esrc0, edst0, esrc1, edst1, esrc2, edst2,
                    deg0, deg1, deg2):
    pos2 = np.full(NUM_DST[2], -1, np.int32)
    pos2[seeds] = np.arange(SEEDS_PER_CORE, dtype=np.int32)
    sel2 = pos2[edst2] >= 0
    es2, ed2g = esrc2[sel2], edst2[sel2]
    l1_extra = np.setdiff1d(np.unique(es2), seeds)
    nfull = (len(l1_extra) // P) * P
    if nfull >= P:
        l1_extra = np.concatenate(
            [_balance(l1_extra[:nfull], deg1, nfull // P), l1_extra[nfull:]]
        )
    l1_out = np.concatenate([seeds, l1_extra])
    n1 = len(l1_out)

    pos1 = np.full(NUM_DST[1], -1, np.int32)
    pos1[l1_out] = np.arange(n1, dtype=np.int32)
    sel1 = pos1[edst1] >= 0
    es1, ed1g = esrc1[sel1], edst1[sel1]
    ed1 = pos1[ed1g].astype(np.int64)
    inv1 = (1.0 / np.maximum(deg1[ed1g], 1.0)).astype(np.float32)
    l0_extra = np.setdiff1d(np.unique(es1), l1_out)
    nfull = (len(l0_extra) // P) * P
    if nfull >= P:
        l0_extra = np.concatenate(
            [_balance(l0_extra[:nfull], deg0, nfull // P), l0_extra[nfull:]]
        )
    l0_out = np.concatenate([l1_out, l0_extra])
    n0 = len(l0_out)

    pos0 = np.full(NUM_DST[0], -1, np.int32)
    pos0[l0_out] = np.arange(n0, dtype=np.int32)
    sel0 = pos0[edst0] >= 0
    es0, ed0g = esrc0[sel0], edst0[sel0]
    ed0 = pos0[ed0g].astype(np.int64)
    inv0 = (1.0 / np.maximum(deg0[ed0g], 1.0)).astype(np.float32)

    ed2 = pos2[ed2g].astype(np.int64)
    inv2 = (1.0 / np.maximum(deg2[ed2g], 1.0)).astype(np.float32)
    es2l = pos1[es2].astype(np.int64)
    es1l = pos0[es1].astype(np.int64)

    return dict(
        seeds=seeds, l1_out=l1_out, l0_out=l0_out, n1=n1, n0=n0,
        e0=(es0.astype(np.int64), ed0, inv0),
        e1=(es1l, ed1, inv1),
        e2=(es2l, ed2, inv2),
    )


def _group_edges_by_tile(es, ed, inv, n_tiles):
    """Per dst-tile: dedup sources, build the dense S' payload.
    Returns per-tile (unique_srcs sorted, W [n_u, 128] f32)."""
    tile = ed // P
    order = np.argsort(tile, kind="stable")
    es, ed, inv, tile = es[order], ed[order], inv[order], tile[order]
    starts = np.searchsorted(tile, np.arange(n_tiles))
    ends = np.searchsorted(tile, np.arange(n_tiles) + 1)
    out = []
    for t in range(n_tiles):
        s, e = starts[t], ends[t]
        u, ii = np.unique(es[s:e], return_inverse=True)
        W = np.zeros((len(u), P), np.float32)
        np.add.at(W, (ii, ed[s:e] - t * P), inv[s:e])
        out.append((u, W))
    return out


class LayerPlan:
    """Continuous slot stream: tile t owns stream slots
    [slot_off[t], slot_off[t]+m[t]).  Chunks are 128-slot groups of the
    stream; a chunk overlapping two tiles gets one masked S' column per
    tile.  pairs lists (tile, chunk) in emission order; the sp column
    index of a pair is its position in that list."""

    def __init__(self, n_tiles, slot_counts, ni):
        self.ni = ni
        self.cpc = ni // P
        self.n_tiles = n_tiles
        self.m = slot_counts
        self.slot_off = np.concatenate([[0], np.cumsum(slot_counts)]).astype(np.int64)
        total = int(self.slot_off[-1])
        self.n_chunks = -(-total // P)
        self.n_calls = -(-self.n_chunks // self.cpc)
        self.n_chunks_pad = self.n_calls * self.cpc
        self.pairs = []
        self.tile_pairs = []  # per tile: list of (sp_col, chunk)
        for t in range(n_tiles):
            lo, hi = int(self.slot_off[t]), int(self.slot_off[t + 1])
            ch1 = (hi - 1) // P if hi > lo else lo // P
            tp = []
            for ch in range(lo // P, ch1 + 1):
                tp.append((len(self.pairs), ch))
                self.pairs.append((t, ch))
            self.tile_pairs.append(tp)
        self.n_sp_cols = len(self.pairs)
        self.gidx = []  # [NCORES][128, n_chunks_pad] int64 table rows
        self.wmat = []  # [NCORES][128, n_sp_cols, 128] f32
        self.call_base = None  # [n_calls] int64, uniform across cores


def _plan_layer(per_core_tiles, n_tiles, ni):
    m = [
        max(1, max(len(per_core_tiles[c][t][0]) for c in range(NCORES)))
        for t in range(n_tiles)
    ]
    return LayerPlan(n_tiles, m, ni)


def _fill_plan(plan, per_core_tiles, pad_row):
    """gidx: per-core slot stream of table rows (absolute; layer 0 is
    remapped to band-local afterwards).  wmat: per-sp-col masked payload."""
    total_pad = plan.n_chunks_pad * P
    for c in range(NCORES):
        stream = np.zeros(total_pad, np.int64)
        for t in range(plan.n_tiles):
            lo, hi = int(plan.slot_off[t]), int(plan.slot_off[t + 1])
            u, _ = per_core_tiles[c][t]
            stream[lo : lo + len(u)] = u
            stream[lo + len(u) : hi] = pad_row[c][t]
        tail = int(plan.slot_off[-1])
        stream[tail:] = pad_row[c][plan.n_tiles - 1]
        plan.gidx.append(stream.reshape(plan.n_chunks_pad, P).T.copy())

        wmat = np.zeros((P, plan.n_sp_cols, P), np.float32)
        for t in range(plan.n_tiles):
            lo = int(plan.slot_off[t])
            u, W = per_core_tiles[c][t]
            for sp_col, ch in plan.tile_pairs[t]:
                s0 = ch * P
                a = max(s0, lo)
                b = min(s0 + P, lo + len(u))
                if a < b:
                    wmat[a - s0 : b - s0, sp_col, :] = W[a - lo : b - lo]
        plan.wmat.append(wmat)


def build_host(inputs):
    esrc0 = np.asarray(inputs["esrc0"]).astype(np.int64)
    edst0 = np.asarray(inputs["edst0"]).astype(np.int64)
    esrc1 = np.asarray(inputs["esrc1"]).astype(np.int64)
    edst1 = np.asarray(inputs["edst1"]).astype(np.int64)
    esrc2 = np.asarray(inputs["esrc2"]).astype(np.int64)
    edst2 = np.asarray(inputs["edst2"]).astype(np.int64)
    x = np.asarray(inputs["x"], dtype=np.float32)

    deg0 = np.bincount(edst0, minlength=NUM_DST[0]).astype(np.float32)
    deg1 = np.bincount(edst1, minlength=NUM_DST[1]).astype(np.float32)
    deg2 = np.bincount(edst2, minlength=NUM_DST[2]).astype(np.float32)

    seed_groups = _seed_partition(esrc0, edst0, esrc1, edst1, esrc2, edst2,
                                  deg0, deg1)
    blocks = [
        _block_for_core(seed_groups[c], esrc0, edst0, esrc1, edst1, esrc2,
                        edst2, deg0, deg1, deg2)
        for c in range(NCORES)
    ]

    n0_pad = max(-(-b["n0"] // P) for b in blocks) * P
    n1_pad = max(-(-b["n1"] // P) for b in blocks) * P
    T0, T1, T2 = n0_pad // P, n1_pad // P, 1

    tiles0 = [_group_edges_by_tile(*b["e0"], T0) for b in blocks]
    tiles1 = [_group_edges_by_tile(*b["e1"], T1) for b in blocks]
    tiles2 = [_group_edges_by_tile(*b["e2"], T2) for b in blocks]

    plan0 = _plan_layer(tiles0, T0, NI_L[0])
    plan1 = _plan_layer(tiles1, T1, NI_L[1])
    plan2 = _plan_layer(tiles2, T2, NI_L[2])

    l0_padded = []
    for b in blocks:
        v = np.zeros(T0 * P, np.int64)
        v[: b["n0"]] = b["l0_out"]
        v[b["n0"] :] = b["l0_out"][0]
        l0_padded.append(v)
    pad0 = [[l0_padded[c][t * P] for t in range(T0)] for c in range(NCORES)]
    _fill_plan(plan0, tiles0, pad0)
    padL = lambda T: [[t * P for t in range(T)] for _ in range(NCORES)]
    _fill_plan(plan1, tiles1, padL(T1))
    _fill_plan(plan2, tiles2, padL(T2))

    # ---- layer 0: banded compact x table (uniform band offsets) ----
    # band t = [tile t's 128 dst rows (slot order) | m[t] source slots]
    band_size = P + np.asarray(plan0.m, np.int64)
    band_start = np.concatenate([[0], np.cumsum(band_size)])

    for c in range(NCORES):
        new = np.zeros_like(plan0.gidx[c])
        for t in range(T0):
            u, _ = tiles0[c][t]
            lo, hi = int(plan0.slot_off[t]), int(plan0.slot_off[t + 1])
            idxs = np.arange(lo, hi)
            rel = idxs - lo
            rows = np.where(rel < len(u), band_start[t] + P + rel, band_start[t])
            new[idxs % P, idxs // P] = rows
        tail = int(plan0.slot_off[-1])
        if plan0.n_chunks_pad * P > tail:
            idxs = np.arange(tail, plan0.n_chunks_pad * P)
            new[idxs % P, idxs // P] = band_start[T0 - 1]
        plan0.gidx[c] = new

    # per-call window base = band start of the first tile overlapping the
    # call's first chunk
    ch_first_slot = np.arange(plan0.n_chunks_pad, dtype=np.int64) * P
    chunk_tile = np.minimum(
        np.searchsorted(plan0.slot_off, ch_first_slot, side="right") - 1,
        T0 - 1,
    )
    cpc0 = plan0.cpc
    plan0.call_base = np.array(
        [band_start[chunk_tile[k * cpc0]] for k in range(plan0.n_calls)], np.int64
    )
    for c in range(NCORES):
        g = plan0.gidx[c]
        for k in range(plan0.n_calls):
            sl = g[:, k * cpc0 : (k + 1) * cpc0]
            assert sl.min() >= plan0.call_base[k], (c, k)
            assert sl.max() < plan0.call_base[k] + WINDOW, (c, k)
    plan1.call_base = np.zeros(plan1.n_calls, np.int64)
    plan2.call_base = np.zeros(plan2.n_calls, np.int64)
    assert n0_pad <= WINDOW and n1_pad <= WINDOW

    bf16 = _bf16()
    x16 = x.astype(bf16)
    xc_len_pad = -(-int(band_start[T0]) // P) * P
    xcs = []
    for c in range(NCORES):
        t = np.zeros((xc_len_pad, FEAT), bf16)
        for tt in range(T0):
            o = int(band_start[tt])
            t[o : o + P] = x16[l0_padded[c][tt * P : (tt + 1) * P]]
            u, _ = tiles0[c][tt]
            t[o + P : o + P + len(u)] = x16[u]
        xcs.append(t)

    return dict(
        plans=(plan0, plan1, plan2),
        band_start=band_start,
        T=(T0, T1, T2),
        n0_pad=n0_pad,
        n1_pad=n1_pad,
        xc_len_pad=xc_len_pad,
        xcs=xcs,
        blocks=blocks,
        weights=tuple(
            (
                np.asarray(inputs[f"W_self{l}"], np.float32),
                np.asarray(inputs[f"W_neigh{l}"], np.float32),
                np.asarray(inputs[f"b{l}"], np.float32),
            )
            for l in range(3)
        ),
    )


# ---------------------------------------------------------------------------
# Numpy simulation of the device kernel (validation aid; fp32 stand-in)
# ---------------------------------------------------------------------------


def simulate_core(meta, c):
    table = meta["xcs"][c].astype(np.float32)
    band_start = meta["band_start"]
    for l, plan in enumerate(meta["plans"]):
        ws, wn, b = meta["weights"][l]
        out = np.zeros((plan.n_tiles * P, OUTW[l]), np.float32)
        for t in range(plan.n_tiles):
            hd_off = band_start[t] if l == 0 else t * P
            hd = table[hd_off : hd_off + P]
            aggT = np.zeros((FEAT, P), np.float32)
            for sp_col, ch in plan.tile_pairs[t]:
                msgs = table[plan.gidx[c][:, ch]]
                aggT += msgs.T @ plan.wmat[c][:, sp_col, :]
            y = hd @ ws + aggT.T @ wn + b
            if l < 2:
                y = np.maximum(y, 0.0)
            out[t * P : (t + 1) * P] = y
        table = out
    return table[:SEEDS_PER_CORE]


# ---------------------------------------------------------------------------
# Device kernel
# ---------------------------------------------------------------------------


def _wrap_idx16(plan, c):
    ni, cpc = plan.ni, plan.cpc
    rel = plan.gidx[c] - np.repeat(plan.call_base, cpc)[None, :]
    n_calls = plan.n_calls
    out = np.zeros((P, n_calls * ni // 16), np.int16)
    for k in range(n_calls):
        flat = rel[:, k * cpc : (k + 1) * cpc].T.reshape(-1)
        w = flat.reshape(ni // 16, 16).T.astype(np.int16)
        out[:16, k * (ni // 16) : (k + 1) * (ni // 16)] = w
    for rep in range(1, 8):
        out[rep * 16 : (rep + 1) * 16] = out[:16]
    return out


def run_device(meta, trace=False):
    import concourse.bacc as bacc
    import concourse.tile as tile
    import concourse.mybir as mybir
    from concourse.bass_utils import run_bass_kernel_spmd

    plans = meta["plans"]
    f32 = mybir.dt.float32
    b16 = mybir.dt.bfloat16

    nc = bacc.Bacc("TRN2", target_bir_lowering=False, debug=False, num_devices=NCORES)

    xc = nc.dram_tensor("xc", [meta["xc_len_pad"], FEAT], b16, kind="ExternalInput")
    ident_d = nc.dram_tensor("ident", [P, P], b16, kind="ExternalInput")
    ones_d = nc.dram_tensor("ones", [1, P], b16, kind="ExternalInput")
    h1buf = nc.dram_tensor("h1buf", [meta["n0_pad"], FEAT], b16)
    h2buf = nc.dram_tensor("h2buf", [meta["n1_pad"], FEAT], b16)
    out_d = nc.dram_tensor("out", [SEEDS_PER_CORE, OUTW[2]], f32, kind="ExternalOutput")

    idx_d, sp_d, w_d = [], [], []
    for l, plan in enumerate(plans):
        idx_d.append(
            nc.dram_tensor(f"gidx{l}", [P, plan.n_calls * plan.ni // 16],
                           mybir.dt.int16, kind="ExternalInput")
        )
        sp_d.append(
            nc.dram_tensor(f"sp{l}", [P, plan.n_sp_cols * P], b16,
                           kind="ExternalInput")
        )
        w_d.append(
            (
                nc.dram_tensor(f"ws{l}", [FEAT, OUTW[l]], b16, kind="ExternalInput"),
                nc.dram_tensor(f"wn{l}", [FEAT, OUTW[l]], b16, kind="ExternalInput"),
                nc.dram_tensor(f"bias{l}", [1, OUTW[l]], b16, kind="ExternalInput"),
            )
        )

    tables = [xc, h1buf, h2buf]
    dests = [h1buf, h2buf, out_d]

    with tile.TileContext(nc) as tc:
        with (
            tc.tile_pool(name="const", bufs=1) as cpool,
            tc.tile_pool(name="msgs", bufs=5) as mpool,
            tc.tile_pool(name="sel", bufs=5) as spool,
            tc.tile_pool(name="acc", bufs=2) as apool,
            tc.tile_pool(name="outp", bufs=3) as opool,
            tc.tile_pool(name="pagg", bufs=2, space="PSUM") as pa,
            tc.tile_pool(name="py", bufs=2, space="PSUM") as pypool,
        ):
            ident_t = cpool.tile([P, P], b16, tag="ident")
            nc.sync.dma_start(out=ident_t[:], in_=ident_d[:])
            ones_t = cpool.tile([1, P], b16, tag="ones")
            nc.sync.dma_start(out=ones_t[:], in_=ones_d[:])
            idx_ts, ws_ts, wn_ts, bias_ts = [], [], [], []
            for l, plan in enumerate(plans):
                outw = OUTW[l]
                idx_t = cpool.tile(list(idx_d[l].shape), mybir.dt.int16, tag=f"idx{l}")
                nc.sync.dma_start(out=idx_t[:], in_=idx_d[l][:])
                idx_ts.append(idx_t)
                wst, wnt = [], []
                for k in range(2):
                    w = cpool.tile([P, outw], b16, tag=f"ws{l}_{k}")
                    nc.sync.dma_start(out=w[:], in_=w_d[l][0][k * P : (k + 1) * P, :])
                    wst.append(w)
                    w = cpool.tile([P, outw], b16, tag=f"wn{l}_{k}")
                    nc.sync.dma_start(out=w[:], in_=w_d[l][1][k * P : (k + 1) * P, :])
                    wnt.append(w)
                ws_ts.append(wst)
                wn_ts.append(wnt)
                bias_t = cpool.tile([1, outw], b16, tag=f"bias{l}")
                nc.sync.dma_start(out=bias_t[:], in_=w_d[l][2][:])
                bias_ts.append(bias_t)

            for l, plan in enumerate(plans):
                outw = OUTW[l]
                table, dest = tables[l], dests[l]
                ws_t, wn_t, bias_t, idx_t = ws_ts[l], wn_ts[l], bias_ts[l], idx_ts[l]
                ni, cpc = plan.ni, plan.cpc

                call_tiles = []
                for k in range(plan.n_calls):
                    mt = mpool.tile([P, cpc * FEAT], b16, tag=f"msgs{l}")
                    base = int(plan.call_base[k])
                    hi = min(base + WINDOW, table.shape[0])
                    nc.gpsimd.dma_gather(
                        out_ap=mt[:, : cpc * FEAT].rearrange(
                            "p (g d) -> p g d", g=cpc
                        ),
                        in_ap=table[base:hi, :],
                        idxs_ap=idx_t[:, k * (ni // 16) : (k + 1) * (ni // 16)],
                        num_idxs=ni,
                        num_idxs_reg=ni,
                        elem_size=FEAT,
                        single_packet=False,
                    )
                    call_tiles.append(mt)

                SPG = cpc  # sp columns per slab tile
                n_slabs = -(-plan.n_sp_cols // SPG)
                sp_tiles = []
                for k in range(n_slabs):
                    c0 = k * SPG * P
                    c1 = min((k + 1) * SPG * P, plan.n_sp_cols * P)
                    st = spool.tile([P, SPG * P], b16, tag=f"sp{l}")
                    nc.sync.dma_start(out=st[:, : c1 - c0], in_=sp_d[l][:, c0:c1])
                    sp_tiles.append(st)

                def msg_slice(ch, f0, f1):
                    k, j = divmod(ch, cpc)
                    return call_tiles[k][:, j * FEAT + f0 : j * FEAT + f1]

                def sp_slice(col):
                    k, j = divmod(col, SPG)
                    return sp_tiles[k][:, j * P : (j + 1) * P]

                band_start = meta["band_start"]
                for t in range(plan.n_tiles):
                    hd_off = int(band_start[t]) if l == 0 else t * P
                    hd = opool.tile([P, FEAT], b16, tag="hd")
                    nc.sync.dma_start(out=hd[:], in_=table[hd_off : hd_off + P, :])
                    # pc{k}[:, 0:128] = meanT f-chunk k; [:, 128:256] = h_dstT
                    pc0 = pa.tile([P, 2 * P], f32, tag="pc0")
                    pc1 = pa.tile([P, 2 * P], f32, tag="pc1")
                    nc.tensor.matmul(pc0[:, P : 2 * P], lhsT=hd[:, 0:P],
                                     rhs=ident_t[:], start=True, stop=True)
                    nc.tensor.matmul(pc1[:, P : 2 * P], lhsT=hd[:, P : 2 * P],
                                     rhs=ident_t[:], start=True, stop=True)
                    pairs = plan.tile_pairs[t]
                    for i, (sp_col, ch) in enumerate(pairs):
                        st, sp = (i == 0), (i == len(pairs) - 1)
                        nc.tensor.matmul(pc0[:, 0:P], lhsT=msg_slice(ch, 0, P),
                                         rhs=sp_slice(sp_col), start=st, stop=sp)
                        nc.tensor.matmul(pc1[:, 0:P], lhsT=msg_slice(ch, P, 2 * P),
                                         rhs=sp_slice(sp_col), start=st, stop=sp)
                    ac0 = apool.tile([P, 2 * P], b16, tag="ac0")
                    nc.vector.tensor_copy(out=ac0[:], in_=pc0[:])
                    ac1 = apool.tile([P, 2 * P], b16, tag="ac1")
                    nc.vector.tensor_copy(out=ac1[:], in_=pc1[:])
                    y = pypool.tile([P, outw], f32, tag="y")
                    nc.tensor.matmul(y[:], lhsT=ac0[:, 0:P], rhs=wn_t[0][:],
                                     start=True, stop=False)
                    nc.tensor.matmul(y[:], lhsT=ac1[:, 0:P], rhs=wn_t[1][:],
                                     start=False, stop=False)
                    nc.tensor.matmul(y[:], lhsT=ac0[:, P : 2 * P], rhs=ws_t[0][:],
                                     start=False, stop=False)
                    nc.tensor.matmul(y[:], lhsT=ac1[:, P : 2 * P], rhs=ws_t[1][:],
                                     start=False, stop=False)
                    nc.tensor.matmul(y[:], lhsT=ones_t[0:1, :], rhs=bias_t[0:1, :],
                                     start=False, stop=True)
                    if l < 2:
                        o2 = opool.tile([P, outw], b16, tag="o2")
                        nc.scalar.activation(
                            out=o2[:], in_=y[:],
                            func=mybir.ActivationFunctionType.Relu,
                        )
                        nc.sync.dma_start(out=dest[t * P : (t + 1) * P, :], in_=o2[:])
                    else:
                        o = opool.tile([P, outw], f32, tag="o")
                        nc.vector.tensor_copy(out=o[:], in_=y[:])
                        nc.sync.dma_start(out=dest[:], in_=o[0:SEEDS_PER_CORE, :])

    nc.compile()

    in_maps = []
    bf16 = _bf16()
    eye16 = np.eye(P, dtype=bf16)
    for c in range(NCORES):
        m = dict(xc=meta["xcs"][c], ident=eye16,
                 ones=np.ones((1, P), dtype=bf16))
        for l, plan in enumerate(plans):
            m[f"gidx{l}"] = _wrap_idx16(plan, c)
            m[f"sp{l}"] = np.ascontiguousarray(
                plan.wmat[c].astype(bf16).reshape(P, plan.n_sp_cols * P)
            )
            ws, wn, b = meta["weights"][l]
            m[f"ws{l}"] = np.ascontiguousarray(ws.astype(bf16))
            m[f"wn{l}"] = np.ascontiguousarray(wn.astype(bf16))
            m[f"bias{l}"] = np.ascontiguousarray(b[None, :].astype(bf16))
        in_maps.append(m)

    res = run_bass_kernel_spmd(
        nc, in_maps, core_ids=list(range(NCORES)), trace=trace
    )
    return [res.results[c]["out"] for c in range(NCORES)], res


def assemble(meta, outs):
    full = np.zeros((NUM_DST[2], OUTW[2]), np.float32)
    for c in range(NCORES):
        full[meta["blocks"][c]["seeds"]] = outs[c]
    return full


def kernel(**inputs) -> np.ndarray:
    meta = build_host(inputs)
    outs, _ = run_device(meta)
    return assemble(meta, outs)


# revision 22
# speedup vs baseline: 2.8191x; 2.2186x over previous
"""DistSAGE 3-layer GraphSAGE forward on 8 TRN2 NeuronCores (Bass/Tile).

Strategy (graph/data parallel, per the DistSAGE recipe):
  - Partition the 512 seed nodes across 8 cores (64 each, LPT-balanced by
    an additive 2-hop cost estimate); build per-core dependency-driven
    blocks on the host (standard DGL block construction, pure index math).
    No inter-core communication; weights replicated.
  - Row-shard the feature table: each core receives a compact bf16 table
    holding only the x rows its block touches, organized as per-dst-tile
    bands [128 h_dst rows | the tile's unique source rows], stored
    PRE-INTERLEAVED in SBUF layout (row g*128+p at [partition p, group g])
    so every layer-0 tile's working set loads as one contiguous dense DMA
    at line rate -- no per-row gather descriptors at all.
  - Per 128-dst tile: meanT[f,d] += msgs_chunk.T @ S' accumulates on the
    TensorEngine in PSUM, where S'[p,d] = sum of 1/deg[d] over edges
    (band_row -> d) is a host-baked bf16 mask streamed on the second HWDGE
    ring; h_dstT comes from 2 identity matmuls of the band's dst group.
    Then Y[d,:] = meanT.T@W_neigh + h_dstT.T@W_self + 1s^T@bias
    (PSUM-accumulated bf16 matmuls), ReLU straight from PSUM, DMA the
    bf16 tile to DRAM.
  - Layers 1/2 read their (runtime-produced) h buffers with the custom
    dma_gather ucode (int16 indices, 2048/call, small tail calls to keep
    the latency-bound drain short at layer boundaries).
"""

import heapq

import numpy as np

P = 128
NCORES = 8
NUM_DST = (61952, 5632, 512)
FEAT = 256
OUTW = (256, 256, 19)
SEEDS_PER_CORE = NUM_DST[2] // NCORES  # 64
WINDOW = 32768
NI_GATHER = 2048  # dma_gather indices per call (layers 1/2)


def _bf16():
    import ml_dtypes

    return ml_dtypes.bfloat16


# ---------------------------------------------------------------------------
# Host-side block construction
# ---------------------------------------------------------------------------


def _balance(ids, deg, n_buckets):
    """LPT bin-packing: reorder ids so consecutive 128-groups have ~equal
    total degree (only full 128-groups are balanced)."""
    if n_buckets <= 1 or len(ids) < n_buckets * P:
        return ids
    order = np.argsort(-deg[ids], kind="stable")
    heap = [(0.0, b, 0) for b in range(n_buckets)]
    heapq.heapify(heap)
    buckets = [[] for _ in range(n_buckets)]
    for i in order:
        load, b, cnt = heapq.heappop(heap)
        buckets[b].append(ids[i])
        cnt += 1
        if cnt < P:
            heapq.heappush(heap, (load + deg[ids[i]], b, cnt))
    return np.concatenate([np.asarray(b, dtype=ids.dtype) for b in buckets])


def _seed_partition(esrc0, edst0, esrc1, edst1, esrc2, edst2, deg0, deg1):
    """LPT-balance seeds across cores by an additive 2-hop cost estimate."""
    h = np.zeros(NUM_DST[1], np.float64)
    np.add.at(h, edst1, deg0[esrc1].astype(np.float64))
    cost = np.zeros(NUM_DST[2], np.float64)
    np.add.at(cost, edst2, h[esrc2] + deg1[esrc2].astype(np.float64))
    order = np.argsort(-cost, kind="stable")
    heap = [(0.0, cc, 0) for cc in range(NCORES)]
    heapq.heapify(heap)
    groups = [[] for _ in range(NCORES)]
    for s in order:
        load, cc, cnt = heapq.heappop(heap)
        groups[cc].append(s)
        cnt += 1
        if cnt < SEEDS_PER_CORE:
            heapq.heappush(heap, (load + cost[s], cc, cnt))
    return [np.array(g, dtype=np.int64) for g in groups]


def _block_for_core(seeds, esrc0, edst0, esrc1, edst1, esrc2, edst2,
                    deg0, deg1, deg2):
    pos2 = np.full(NUM_DST[2], -1, np.int32)
    pos2[seeds] = np.arange(SEEDS_PER_CORE, dtype=np.int32)
    sel2 = pos2[edst2] >= 0
    es2, ed2g = esrc2[sel2], edst2[sel2]
    l1_extra = np.setdiff1d(np.unique(es2), seeds)
    nfull = (len(l1_extra) // P) * P
    if nfull >= P:
        l1_extra = np.concatenate(
            [_balance(l1_extra[:nfull], deg1, nfull // P), l1_extra[nfull:]]
        )
    l1_out = np.concatenate([seeds, l1_extra])
    n1 = len(l1_out)

    pos1 = np.full(NUM_DST[1], -1, np.int32)
    pos1[l1_out] = np.arange(n1, dtype=np.int32)
    sel1 = pos1[edst1] >= 0
    es1, ed1g = esrc1[sel1], edst1[sel1]
    ed1 = pos1[ed1g].astype(np.int64)
    inv1 = (1.0 / np.maximum(deg1[ed1g], 1.0)).astype(np.float32)
    l0_extra = np.setdiff1d(np.unique(es1), l1_out)
    nfull = (len(l0_extra) // P) * P
    if nfull >= P:
        l0_extra = np.concatenate(
            [_balance(l0_extra[:nfull], deg0, nfull // P), l0_extra[nfull:]]
        )
    l0_out = np.concatenate([l1_out, l0_extra])
    n0 = len(l0_out)

    pos0 = np.full(NUM_DST[0], -1, np.int32)
    pos0[l0_out] = np.arange(n0, dtype=np.int32)
    sel0 = pos0[edst0] >= 0
    es0, ed0g = esrc0[sel0], edst0[sel0]
    ed0 = pos0[ed0g].astype(np.int64)
    inv0 = (1.0 / np.maximum(deg0[ed0g], 1.0)).astype(np.float32)

    ed2 = pos2[ed2g].astype(np.int64)
    inv2 = (1.0 / np.maximum(deg2[ed2g], 1.0)).astype(np.float32)
    es2l = pos1[es2].astype(np.int64)
    es1l = pos0[es1].astype(np.int64)

    return dict(
        seeds=seeds, l1_out=l1_out, l0_out=l0_out, n1=n1, n0=n0,
        e0=(es0.astype(np.int64), ed0, inv0),
        e1=(es1l, ed1, inv1),
        e2=(es2l, ed2, inv2),
    )


def _group_edges_by_tile(es, ed, inv, n_tiles):
    """Per dst-tile: dedup sources, build the dense S' payload.
    Returns per-tile (unique_srcs sorted, W [n_u, 128] f32)."""
    tile = ed // P
    order = np.argsort(tile, kind="stable")
    es, ed, inv, tile = es[order], ed[order], inv[order], tile[order]
    starts = np.searchsorted(tile, np.arange(n_tiles))
    ends = np.searchsorted(tile, np.arange(n_tiles) + 1)
    out = []
    for t in range(n_tiles):
        s, e = starts[t], ends[t]
        u, ii = np.unique(es[s:e], return_inverse=True)
        W = np.zeros((len(u), P), np.float32)
        np.add.at(W, (ii, ed[s:e] - t * P), inv[s:e])
        out.append((u, W))
    return out


class GatherPlan:
    """Layers 1/2: continuous slot stream gathered via dma_gather.
    Tile t owns stream slots [slot_off[t], slot_off[t]+m[t]); chunks are
    128-slot groups; a chunk overlapping two tiles gets one masked S'
    column per tile."""

    def __init__(self, n_tiles, slot_counts, ni):
        self.ni = ni
        self.cpc = ni // P
        self.n_tiles = n_tiles
        self.m = slot_counts
        self.slot_off = np.concatenate([[0], np.cumsum(slot_counts)]).astype(np.int64)
        total = int(self.slot_off[-1])
        self.n_chunks = -(-total // P)
        # full-size calls + small tail calls (short drain at layer end)
        TAIL = 4
        full = max(0, (self.n_chunks - self.cpc) // self.cpc)
        rem = self.n_chunks - full * self.cpc
        self.call_sizes = [self.cpc] * full + [TAIL] * (-(-rem // TAIL))
        self.n_chunks_pad = sum(self.call_sizes)
        self.n_calls = len(self.call_sizes)
        self.call_chunk_off = np.concatenate(
            [[0], np.cumsum(self.call_sizes)]
        ).astype(np.int64)
        self.pairs = []
        self.tile_pairs = []  # per tile: list of (sp_col, chunk)
        for t in range(n_tiles):
            lo, hi = int(self.slot_off[t]), int(self.slot_off[t + 1])
            ch1 = (hi - 1) // P if hi > lo else lo // P
            tp = []
            for ch in range(lo // P, ch1 + 1):
                tp.append((len(self.pairs), ch))
                self.pairs.append((t, ch))
            self.tile_pairs.append(tp)
        self.n_sp_cols = len(self.pairs)
        self.gidx = []  # [NCORES][128, n_chunks_pad] int64 table rows
        self.wmat = []  # [NCORES][128, n_sp_cols, 128] f32
        self.call_base = None


def _plan_gather(per_core_tiles, n_tiles, ni):
    m = [
        max(1, max(len(per_core_tiles[c][t][0]) for c in range(NCORES)))
        for t in range(n_tiles)
    ]
    return GatherPlan(n_tiles, m, ni)


def _fill_gather(plan, per_core_tiles, pad_row):
    total_pad = plan.n_chunks_pad * P
    for c in range(NCORES):
        stream = np.zeros(total_pad, np.int64)
        for t in range(plan.n_tiles):
            lo, hi = int(plan.slot_off[t]), int(plan.slot_off[t + 1])
            u, _ = per_core_tiles[c][t]
            stream[lo : lo + len(u)] = u
            stream[lo + len(u) : hi] = pad_row[c][t]
        tail = int(plan.slot_off[-1])
        stream[tail:] = pad_row[c][plan.n_tiles - 1]
        plan.gidx.append(stream.reshape(plan.n_chunks_pad, P).T.copy())

        wmat = np.zeros((P, plan.n_sp_cols, P), np.float32)
        for t in range(plan.n_tiles):
            lo = int(plan.slot_off[t])
            u, W = per_core_tiles[c][t]
            for sp_col, ch in plan.tile_pairs[t]:
                s0 = ch * P
                a = max(s0, lo)
                b = min(s0 + P, lo + len(u))
                if a < b:
                    wmat[a - s0 : b - s0, sp_col, :] = W[a - lo : b - lo]
        plan.wmat.append(wmat)


class BandPlan:
    """Layer 0: per-tile dense bands, pre-interleaved.  Tile t's band =
    group 0 (h_dst rows) + groups 1..K[t] (source chunks); group g sits at
    xc2[:, (goff[t]+g)*256 : ...]."""

    def __init__(self, n_tiles, src_counts):
        self.n_tiles = n_tiles
        self.m = src_counts  # real (max-over-core) source count per tile
        self.K = [max(1, -(-m // P)) for m in src_counts]
        self.goff = np.concatenate(
            [[0], np.cumsum([1 + k for k in self.K])]
        ).astype(np.int64)
        self.n_groups = int(self.goff[-1])
        self.n_sp_cols = sum(self.K)
        self.sp_off = np.concatenate([[0], np.cumsum(self.K)]).astype(np.int64)
        self.wmat = []  # [NCORES][128, n_sp_cols, 128] f32


def build_host(inputs):
    esrc0 = np.asarray(inputs["esrc0"]).astype(np.int64)
    edst0 = np.asarray(inputs["edst0"]).astype(np.int64)
    esrc1 = np.asarray(inputs["esrc1"]).astype(np.int64)
    edst1 = np.asarray(inputs["edst1"]).astype(np.int64)
    esrc2 = np.asarray(inputs["esrc2"]).astype(np.int64)
    edst2 = np.asarray(inputs["edst2"]).astype(np.int64)
    x = np.asarray(inputs["x"], dtype=np.float32)

    deg0 = np.bincount(edst0, minlength=NUM_DST[0]).astype(np.float32)
    deg1 = np.bincount(edst1, minlength=NUM_DST[1]).astype(np.float32)
    deg2 = np.bincount(edst2, minlength=NUM_DST[2]).astype(np.float32)

    seed_groups = _seed_partition(esrc0, edst0, esrc1, edst1, esrc2, edst2,
                                  deg0, deg1)
    blocks = [
        _block_for_core(seed_groups[c], esrc0, edst0, esrc1, edst1, esrc2,
                        edst2, deg0, deg1, deg2)
        for c in range(NCORES)
    ]

    n0_pad = max(-(-b["n0"] // P) for b in blocks) * P
    n1_pad = max(-(-b["n1"] // P) for b in blocks) * P
    T0, T1, T2 = n0_pad // P, n1_pad // P, 1

    tiles0 = [_group_edges_by_tile(*b["e0"], T0) for b in blocks]
    tiles1 = [_group_edges_by_tile(*b["e1"], T1) for b in blocks]
    tiles2 = [_group_edges_by_tile(*b["e2"], T2) for b in blocks]

    # ---- layer 0: band plan + pre-interleaved compact tables ----
    plan0 = BandPlan(
        T0,
        [max(len(tiles0[c][t][0]) for c in range(NCORES)) for t in range(T0)],
    )
    l0_padded = []
    for b in blocks:
        v = np.zeros(T0 * P, np.int64)
        v[: b["n0"]] = b["l0_out"]
        v[b["n0"] :] = b["l0_out"][0]
        l0_padded.append(v)

    bf16 = _bf16()
    x16 = x.astype(bf16)
    xc2s = []
    for c in range(NCORES):
        xr = np.zeros((P, plan0.n_groups, FEAT), bf16)
        wmat = np.zeros((P, plan0.n_sp_cols, P), np.float32)
        for t in range(T0):
            g0 = int(plan0.goff[t])
            xr[:, g0, :] = x16[l0_padded[c][t * P : (t + 1) * P]]
            u, W = tiles0[c][t]
            rows = x16[u]
            for k in range(plan0.K[t]):
                a, b = k * P, min((k + 1) * P, len(u))
                if a < b:
                    xr[: b - a, g0 + 1 + k, :] = rows[a:b]
                    wmat[: b - a, int(plan0.sp_off[t]) + k, :] = W[a:b]
        xc2s.append(np.ascontiguousarray(xr.reshape(P, plan0.n_groups * FEAT)))
        plan0.wmat.append(wmat)

    # ---- layers 1/2: gather plans ----
    plan1 = _plan_gather(tiles1, T1, NI_GATHER)
    plan2 = _plan_gather(tiles2, T2, min(NI_GATHER, 1024))
    padL = lambda T: [[t * P for t in range(T)] for _ in range(NCORES)]
    _fill_gather(plan1, tiles1, padL(T1))
    _fill_gather(plan2, tiles2, padL(T2))
    plan1.call_base = np.zeros(plan1.n_calls, np.int64)
    plan2.call_base = np.zeros(plan2.n_calls, np.int64)
    assert n0_pad <= WINDOW and n1_pad <= WINDOW

    return dict(
        plan0=plan0,
        plans=(plan1, plan2),
        T=(T0, T1, T2),
        n0_pad=n0_pad,
        n1_pad=n1_pad,
        xc2s=xc2s,
        blocks=blocks,
        weights=tuple(
            (
                np.asarray(inputs[f"W_self{l}"], np.float32),
                np.asarray(inputs[f"W_neigh{l}"], np.float32),
                np.asarray(inputs[f"b{l}"], np.float32),
            )
            for l in range(3)
        ),
    )


# ---------------------------------------------------------------------------
# Numpy simulation of the device kernel (validation aid; fp32 stand-in)
# ---------------------------------------------------------------------------


def simulate_core(meta, c):
    plan0 = meta["plan0"]
    xr = meta["xc2s"][c].astype(np.float32).reshape(P, plan0.n_groups, FEAT)

    ws, wn, b = meta["weights"][0]
    table = np.zeros((plan0.n_tiles * P, OUTW[0]), np.float32)
    for t in range(plan0.n_tiles):
        g0 = int(plan0.goff[t])
        hd = xr[:, g0, :]
        aggT = np.zeros((FEAT, P), np.float32)
        for k in range(plan0.K[t]):
            msgs = xr[:, g0 + 1 + k, :]
            aggT += msgs.T @ plan0.wmat[c][:, int(plan0.sp_off[t]) + k, :]
        table[t * P : (t + 1) * P] = np.maximum(hd @ ws + aggT.T @ wn + b, 0.0)

    for li, plan in enumerate(meta["plans"]):
        l = li + 1
        ws, wn, b = meta["weights"][l]
        out = np.zeros((plan.n_tiles * P, OUTW[l]), np.float32)
        for t in range(plan.n_tiles):
            hd = table[t * P : (t + 1) * P]
            aggT = np.zeros((FEAT, P), np.float32)
            for sp_col, ch in plan.tile_pairs[t]:
                msgs = table[plan.gidx[c][:, ch]]
                aggT += msgs.T @ plan.wmat[c][:, sp_col, :]
            y = hd @ ws + aggT.T @ wn + b
            if l < 2:
                y = np.maximum(y, 0.0)
            out[t * P : (t + 1) * P] = y
        table = out
    return table[:SEEDS_PER_CORE]


# ---------------------------------------------------------------------------
# Device kernel
# ---------------------------------------------------------------------------


def _wrap_idx16(plan, c):
    bases = np.zeros(plan.n_chunks_pad, np.int64)
    for k in range(plan.n_calls):
        bases[plan.call_chunk_off[k] : plan.call_chunk_off[k + 1]] = plan.call_base[k]
    rel = plan.gidx[c] - bases[None, :]
    total16 = plan.n_chunks_pad * P // 16
    out = np.zeros((P, total16), np.int16)
    off16 = 0
    for k in range(plan.n_calls):
        a, b = int(plan.call_chunk_off[k]), int(plan.call_chunk_off[k + 1])
        flat = rel[:, a:b].T.reshape(-1)
        w = flat.reshape(len(flat) // 16, 16).T.astype(np.int16)
        out[:16, off16 : off16 + w.shape[1]] = w
        off16 += w.shape[1]
    for rep in range(1, 8):
        out[rep * 16 : (rep + 1) * 16] = out[:16]
    return out


def run_device(meta, trace=False):
    import concourse.bacc as bacc
    import concourse.tile as tile
    import concourse.mybir as mybir
    from concourse.bass_utils import run_bass_kernel_spmd

    plan0 = meta["plan0"]
    plans = meta["plans"]
    f32 = mybir.dt.float32
    b16 = mybir.dt.bfloat16

    nc = bacc.Bacc("TRN2", target_bir_lowering=False, debug=False, num_devices=NCORES)

    xc2 = nc.dram_tensor("xc2", [P, plan0.n_groups * FEAT], b16, kind="ExternalInput")
    sp0_d = nc.dram_tensor("sp0", [P, plan0.n_sp_cols * P], b16, kind="ExternalInput")
    ident_d = nc.dram_tensor("ident", [P, P], b16, kind="ExternalInput")
    ones_d = nc.dram_tensor("ones", [1, P], b16, kind="ExternalInput")
    h1buf = nc.dram_tensor("h1buf", [meta["n0_pad"], FEAT], b16)
    h2buf = nc.dram_tensor("h2buf", [meta["n1_pad"], FEAT], b16)
    out_d = nc.dram_tensor("out", [SEEDS_PER_CORE, OUTW[2]], f32, kind="ExternalOutput")

    idx_d, sp_d = [], []
    for li, plan in enumerate(plans):
        idx_d.append(
            nc.dram_tensor(f"gidx{li + 1}", [P, plan.n_chunks_pad * P // 16],
                           mybir.dt.int16, kind="ExternalInput")
        )
        sp_d.append(
            nc.dram_tensor(f"sp{li + 1}", [P, plan.n_sp_cols * P], b16,
                           kind="ExternalInput")
        )
    w_d = []
    for l in range(3):
        w_d.append(
            (
                nc.dram_tensor(f"ws{l}", [FEAT, OUTW[l]], b16, kind="ExternalInput"),
                nc.dram_tensor(f"wn{l}", [FEAT, OUTW[l]], b16, kind="ExternalInput"),
                nc.dram_tensor(f"bias{l}", [1, OUTW[l]], b16, kind="ExternalInput"),
            )
        )

    with tile.TileContext(nc) as tc:
        with (
            tc.tile_pool(name="const", bufs=1) as cpool,
            tc.tile_pool(name="msgs", bufs=5) as mpool,
            tc.tile_pool(name="sel", bufs=5) as spool,
            tc.tile_pool(name="acc", bufs=2) as apool,
            tc.tile_pool(name="outp", bufs=3) as opool,
            tc.tile_pool(name="pagg", bufs=2, space="PSUM") as pa,
            tc.tile_pool(name="py", bufs=2, space="PSUM") as pypool,
        ):
            ident_t = cpool.tile([P, P], b16, tag="ident")
            nc.sync.dma_start(out=ident_t[:], in_=ident_d[:])
            ones_t = cpool.tile([1, P], b16, tag="ones")
            nc.sync.dma_start(out=ones_t[:], in_=ones_d[:])
            ws_ts, wn_ts, bias_ts = [], [], []
            for l in range(3):
                outw = OUTW[l]
                wst, wnt = [], []
                for k in range(2):
                    w = cpool.tile([P, outw], b16, tag=f"ws{l}_{k}")
                    nc.sync.dma_start(out=w[:], in_=w_d[l][0][k * P : (k + 1) * P, :])
                    wst.append(w)
                    w = cpool.tile([P, outw], b16, tag=f"wn{l}_{k}")
                    nc.sync.dma_start(out=w[:], in_=w_d[l][1][k * P : (k + 1) * P, :])
                    wnt.append(w)
                ws_ts.append(wst)
                wn_ts.append(wnt)
                bias_t = cpool.tile([1, outw], b16, tag=f"bias{l}")
                nc.sync.dma_start(out=bias_t[:], in_=w_d[l][2][:])
                bias_ts.append(bias_t)
            idx_ts = []
            for li, plan in enumerate(plans):
                idx_t = cpool.tile(
                    list(idx_d[li].shape), mybir.dt.int16, tag=f"idx{li + 1}"
                )
                nc.sync.dma_start(out=idx_t[:], in_=idx_d[li][:])
                idx_ts.append(idx_t)

            def tile_tail(l, t, ac0, ac1, dest):
                """Y matmuls + bias + activation + store for one dst tile."""
                outw = OUTW[l]
                y = pypool.tile([P, outw], f32, tag="y")
                nc.tensor.matmul(y[:], lhsT=ac0[:, 0:P], rhs=wn_ts[l][0][:],
                                 start=True, stop=False)
                nc.tensor.matmul(y[:], lhsT=ac1[:, 0:P], rhs=wn_ts[l][1][:],
                                 start=False, stop=False)
                nc.tensor.matmul(y[:], lhsT=ac0[:, P : 2 * P], rhs=ws_ts[l][0][:],
                                 start=False, stop=False)
                nc.tensor.matmul(y[:], lhsT=ac1[:, P : 2 * P], rhs=ws_ts[l][1][:],
                                 start=False, stop=False)
                nc.tensor.matmul(y[:], lhsT=ones_t[0:1, :], rhs=bias_ts[l][0:1, :],
                                 start=False, stop=True)
                if l < 2:
                    o2 = opool.tile([P, outw], b16, tag="o2")
                    nc.scalar.activation(
                        out=o2[:], in_=y[:],
                        func=mybir.ActivationFunctionType.Relu,
                    )
                    nc.sync.dma_start(out=dest[t * P : (t + 1) * P, :], in_=o2[:])
                else:
                    o = opool.tile([P, outw], f32, tag="o")
                    nc.vector.tensor_copy(out=o[:], in_=y[:])
                    nc.sync.dma_start(out=dest[:], in_=o[0:SEEDS_PER_CORE, :])

            # ================= layer 0: dense bands =================
            Kmax = max(plan0.K)
            for t in range(plan0.n_tiles):
                K = plan0.K[t]
                g0 = int(plan0.goff[t])
                bt = mpool.tile([P, (1 + Kmax) * FEAT], b16, tag="band")
                nc.scalar.dma_start(
                    out=bt[:, : (1 + K) * FEAT],
                    in_=xc2[:, g0 * FEAT : (g0 + 1 + K) * FEAT],
                )
                spt = spool.tile([P, Kmax * P], b16, tag="spb")
                so = int(plan0.sp_off[t])
                nc.scalar.dma_start(
                    out=spt[:, : K * P], in_=sp0_d[:, so * P : (so + K) * P]
                )
                pc0 = pa.tile([P, 2 * P], f32, tag="pc0")
                pc1 = pa.tile([P, 2 * P], f32, tag="pc1")
                nc.tensor.matmul(pc0[:, P : 2 * P], lhsT=bt[:, 0:P],
                                 rhs=ident_t[:], start=True, stop=True)
                nc.tensor.matmul(pc1[:, P : 2 * P], lhsT=bt[:, P : 2 * P],
                                 rhs=ident_t[:], start=True, stop=True)
                for k in range(K):
                    st, sp = (k == 0), (k == K - 1)
                    base = (1 + k) * FEAT
                    nc.tensor.matmul(pc0[:, 0:P], lhsT=bt[:, base : base + P],
                                     rhs=spt[:, k * P : (k + 1) * P],
                                     start=st, stop=sp)
                    nc.tensor.matmul(pc1[:, 0:P],
                                     lhsT=bt[:, base + P : base + 2 * P],
                                     rhs=spt[:, k * P : (k + 1) * P],
                                     start=st, stop=sp)
                ac0 = apool.tile([P, 2 * P], b16, tag="ac0")
                nc.vector.tensor_copy(out=ac0[:], in_=pc0[:])
                ac1 = apool.tile([P, 2 * P], b16, tag="ac1")
                nc.vector.tensor_copy(out=ac1[:], in_=pc1[:])
                tile_tail(0, t, ac0, ac1, h1buf)

            # ================= layers 1/2: gather =================
            tables = [h1buf, h2buf]
            dests = [h2buf, out_d]
            for li, plan in enumerate(plans):
                l = li + 1
                table, dest = tables[li], dests[li]
                idx_t = idx_ts[li]
                cpc = plan.cpc

                call_tiles = []
                for k in range(plan.n_calls):
                    a = int(plan.call_chunk_off[k])
                    b2 = int(plan.call_chunk_off[k + 1])
                    sz = b2 - a
                    mt = mpool.tile([P, cpc * FEAT], b16, tag=f"msgs{l}")
                    nc.gpsimd.dma_gather(
                        out_ap=mt[:, : sz * FEAT].rearrange(
                            "p (g d) -> p g d", g=sz
                        ),
                        in_ap=table[:, :],
                        idxs_ap=idx_t[:, a * P // 16 : b2 * P // 16],
                        num_idxs=sz * P,
                        num_idxs_reg=sz * P,
                        elem_size=FEAT,
                        single_packet=False,
                    )
                    call_tiles.append((mt, a))

                SPG = cpc
                n_slabs = -(-plan.n_sp_cols // SPG)
                sp_tiles = []
                for k in range(n_slabs):
                    c0 = k * SPG * P
                    c1 = min((k + 1) * SPG * P, plan.n_sp_cols * P)
                    st = spool.tile([P, SPG * P], b16, tag=f"sp{l}")
                    nc.scalar.dma_start(out=st[:, : c1 - c0], in_=sp_d[li][:, c0:c1])
                    sp_tiles.append(st)

                call_of_chunk = np.searchsorted(
                    plan.call_chunk_off, np.arange(plan.n_chunks_pad), side="right"
                ) - 1

                def msg_slice(ch, f0, f1):
                    k = int(call_of_chunk[ch])
                    mt, a = call_tiles[k]
                    j = ch - a
                    return mt[:, j * FEAT + f0 : j * FEAT + f1]

                def sp_slice(col):
                    k, j = divmod(col, SPG)
                    return sp_tiles[k][:, j * P : (j + 1) * P]

                for t in range(plan.n_tiles):
                    hd = opool.tile([P, FEAT], b16, tag="hd")
                    nc.scalar.dma_start(out=hd[:], in_=table[t * P : (t + 1) * P, :])
                    pc0 = pa.tile([P, 2 * P], f32, tag="pc0")
                    pc1 = pa.tile([P, 2 * P], f32, tag="pc1")
                    nc.tensor.matmul(pc0[:, P : 2 * P], lhsT=hd[:, 0:P],
                                     rhs=ident_t[:], start=True, stop=True)
                    nc.tensor.matmul(pc1[:, P : 2 * P], lhsT=hd[:, P : 2 * P],
                                     rhs=ident_t[:], start=True, stop=True)
                    pairs = plan.tile_pairs[t]
                    for i, (sp_col, ch) in enumerate(pairs):
                        st, sp = (i == 0), (i == len(pairs) - 1)
                        nc.tensor.matmul(pc0[:, 0:P], lhsT=msg_slice(ch, 0, P),
                                         rhs=sp_slice(sp_col), start=st, stop=sp)
                        nc.tensor.matmul(pc1[:, 0:P], lhsT=msg_slice(ch, P, 2 * P),
                                         rhs=sp_slice(sp_col), start=st, stop=sp)
                    ac0 = apool.tile([P, 2 * P], b16, tag="ac0")
                    nc.vector.tensor_copy(out=ac0[:], in_=pc0[:])
                    ac1 = apool.tile([P, 2 * P], b16, tag="ac1")
                    nc.vector.tensor_copy(out=ac1[:], in_=pc1[:])
                    tile_tail(l, t, ac0, ac1, dest)

    nc.compile()

    in_maps = []
    bf16 = _bf16()
    eye16 = np.eye(P, dtype=bf16)
    for c in range(NCORES):
        m = dict(
            xc2=meta["xc2s"][c],
            sp0=np.ascontiguousarray(
                plan0.wmat[c].astype(bf16).reshape(P, plan0.n_sp_cols * P)
            ),
            ident=eye16,
            ones=np.ones((1, P), dtype=bf16),
        )
        for li, plan in enumerate(plans):
            m[f"gidx{li + 1}"] = _wrap_idx16(plan, c)
            m[f"sp{li + 1}"] = np.ascontiguousarray(
                plan.wmat[c].astype(bf16).reshape(P, plan.n_sp_cols * P)
            )
        for l in range(3):
            ws, wn, b = meta["weights"][l]
            m[f"ws{l}"] = np.ascontiguousarray(ws.astype(bf16))
            m[f"wn{l}"] = np.ascontiguousarray(wn.astype(bf16))
            m[f"bias{l}"] = np.ascontiguousarray(b[None, :].astype(bf16))
        in_maps.append(m)

    res = run_bass_kernel_spmd(
        nc, in_maps, core_ids=list(range(NCORES)), trace=trace
    )
    return [res.results[c]["out"] for c in range(NCORES)], res


def assemble(meta, outs):
    full = np.zeros((NUM_DST[2], OUTW[2]), np.float32)
    for c in range(NCORES):
        full[meta["blocks"][c]["seeds"]] = outs[c]
    return full


def kernel(**inputs) -> np.ndarray:
    meta = build_host(inputs)
    outs, _ = run_device(meta)
    return assemble(meta, outs)
